# revision 1
# baseline (speedup 1.0000x reference)
"""Trainium2 Bass kernel for nn_FFTConv: y = tanh(Re(ifft(fft(u)*Ks)) + D*u).

Self-contained: builds constant tables with numpy, shards over 8 NeuronCores
(H-parallel: 32 channels/core), runs a Bass/Tile kernel per core via
run_bass_kernel_spmd, gathers the full output.

Algorithm (per core):
  Prologue:
    G[p,m] = 1/(1 - A_p * WL^m)           (P=64 poles x L=8192 freqs, on DVE/ACT)
    Ks[h]  = BC[h,:] @ G                  (TensorE, -> DRAM scratch, complex)
  Main loop over (h, b-group of 4):  2-stage matmul FFT, L = 128*64
    n = n1 + 128*n2 ; m = k2 + 64*k1
    Y1 = F64 @ u.reshape(64,128)          [k2, n1]
    Y2 = Y1 * T                           twiddle T[k2,n1] = WL^(n1*k2)
    X  = F128 @ Y2.T                      [k1, k2]  (PE transpose between)
    S  = X * Ks[h].reshape(128,64)
    Z1 = conj(F128) @ S                   [o2, k2]
    Z2 = Z1 * conj(TI)                    TI[o2,k2] = WL^(k2*o2)
    xo = Re(conj(F64) @ Z2.T)             [o1, o2]  (PE transpose between)
    y  = tanh(xo/L + D[h]*u)
"""
import os
import sys
import numpy as np

for p in ("/opt/trn_rl_repo", "/root/.axon_site/_ro/trn_rl_repo"):
    if os.path.isdir(p) and p not in sys.path:
        sys.path.append(p)

B, H, L, P = 16, 256, 8192, 64
NCORES = 8
HSH = H // NCORES          # 32 channels per core
GB = 4                     # b-group size (pairs per inner group)
NG = B // GB               # 4 groups per h
F32R = os.environ.get("KERNEL_F32R", "0") == "1"   # reduced-precision fast matmul mode
REPEAT = int(os.environ.get("KERNEL_REPEAT", "1"))  # repeat main loop (timing only)
MIDBUFS = int(os.environ.get("KERNEL_MIDBUFS", "2"))
IOBUFS = int(os.environ.get("KERNEL_IOBUFS", "3"))

_CACHE = {}


def _tables():
    a64 = np.arange(64)
    a128 = np.arange(128)
    th64 = 2 * np.pi * np.outer(a64, a64) / 64.0
    th128 = 2 * np.pi * np.outer(a128, a128) / 128.0
    thT = 2 * np.pi * np.outer(a64, a128) / L       # [k2, n1]
    thTI = 2 * np.pi * np.outer(a128, a64) / L      # [o2, k2]
    t = {
        "f64r": np.cos(th64), "f64i": -np.sin(th64),
        "f128r": np.cos(th128), "f128i": -np.sin(th128), "f128in": np.sin(th128),
        "tr": np.cos(thT), "ti": -np.sin(thT),
        # conj(TI) passed directly: re=cos, im=+sin
        "tir": np.cos(thTI), "tii": np.sin(thTI),
        "i64": np.eye(64), "i128": np.eye(128),
    }
    m = np.arange(L)
    cm = np.cos(2 * np.pi * m / L).reshape(2, 4096)
    sm = np.sin(2 * np.pi * m / L).reshape(2, 4096)
    # pre-replicated across 64 partitions per half: (128, 4096)
    t["cm"] = np.repeat(cm, 64, axis=0)
    t["sm"] = np.repeat(sm, 64, axis=0)
    return {k: v.astype(np.float32) for k, v in t.items()}


def _build(nc_mod):
    """Builds the Bass program (same program for all cores)."""
    bass, tile, mybir, bacc = nc_mod
    dt = mybir.dt
    f32 = dt.float32
    MMDT = dt.float32r if F32R else dt.float32

    def mdt(ap):
        return ap.bitcast(MMDT) if F32R else ap

    nc = bacc.Bacc("TRN2", target_bir_lowering=False, debug=False)
    AF = mybir.ActivationFunctionType
    OP = mybir.AluOpType

    # ---------------- DRAM parameters ----------------
    u_d = nc.declare_dram_parameter("u_sh", [B, HSH, L], f32, isOutput=False)
    y_d = nc.declare_dram_parameter("y_sh", [B, HSH, L], f32, isOutput=True)
    ar_d = nc.declare_dram_parameter("a_re", [2 * P, 1], f32, isOutput=False)
    ai_d = nc.declare_dram_parameter("a_im", [2 * P, 1], f32, isOutput=False)
    bcr_d = nc.declare_dram_parameter("bct_r", [P, HSH], f32, isOutput=False)
    bci_d = nc.declare_dram_parameter("bct_i", [P, HSH], f32, isOutput=False)
    bcin_d = nc.declare_dram_parameter("bct_i_neg", [P, HSH], f32, isOutput=False)
    d_d = nc.declare_dram_parameter("d_sh", [1, HSH], f32, isOutput=False)
    tbl_names = ["f64r", "f64i", "f128r", "f128i", "f128in",
                 "tr", "ti", "tir", "tii", "i64", "i128", "cm", "sm"]
    tbl_shapes = {"f64r": [64, 64], "f64i": [64, 64],
                  "f128r": [128, 128], "f128i": [128, 128], "f128in": [128, 128],
                  "tr": [64, 128], "ti": [64, 128],
                  "tir": [128, 64], "tii": [128, 64],
                  "i64": [64, 64], "i128": [128, 128],
                  "cm": [128, 4096], "sm": [128, 4096]}
    tbl_d = {n: nc.declare_dram_parameter(n, tbl_shapes[n], f32, isOutput=False) for n in tbl_names}

    ksr_d = nc.dram_tensor("ksr_scratch", [HSH, L], f32)
    ksi_d = nc.dram_tensor("ksi_scratch", [HSH, L], f32)

    with tile.TileContext(nc) as tc:
        with (
            tc.tile_pool(name="const", bufs=1) as cpool,
            tc.tile_pool(name="ks", bufs=2) as ksp,
        ):
            # ---------- load constants ----------
            tb = {}
            for n in tbl_names:
                if n in ("cm", "sm"):
                    continue
                tb[n] = cpool.tile(tbl_shapes[n], f32, tag=f"c_{n}", name=f"c_{n}")
                if n in ("f64r", "f64i", "f128r", "f128i", "f128in"):
                    nc.sync.dma_start(mdt(tb[n][:]), mdt(tbl_d[n][:]))
                else:
                    nc.sync.dma_start(tb[n][:], tbl_d[n][:])
            a_re = cpool.tile([128, 1], f32, tag="a_re")
            a_im = cpool.tile([128, 1], f32, tag="a_im")
            nc.sync.dma_start(a_re[:], ar_d[:])
            nc.sync.dma_start(a_im[:], ai_d[:])
            bct_r = cpool.tile([P, HSH], f32, tag="bct_r")
            bct_i = cpool.tile([P, HSH], f32, tag="bct_i")
            bct_in = cpool.tile([P, HSH], f32, tag="bct_in")
            nc.sync.dma_start(mdt(bct_r[:]), mdt(bcr_d[:]))
            nc.sync.dma_start(mdt(bct_i[:]), mdt(bci_d[:]))
            nc.sync.dma_start(mdt(bct_in[:]), mdt(bcin_d[:]))
            # D broadcast to 64 partitions
            d_b = cpool.tile([64, HSH], f32, tag="d_b")
            nc.sync.dma_start(d_b[:], d_d[:].broadcast_to([64, HSH]))

            prologue_pools = (
                tc.tile_pool(name="gwork", bufs=1),
                tc.tile_pool(name="psk", bufs=2, space=bass.MemorySpace.PSUM),
            )
            gpool = prologue_pools[0].__enter__()
            pskp = prologue_pools[1].__enter__()
            # 1 + |A|^2 per partition (stacked twice)
            one_a2 = cpool.tile([128, 1], f32, tag="one_a2")
            t_sq = gpool.tile([128, 1], f32, tag="g_sq")
            nc.scalar.activation(one_a2[:], a_re[:], AF.Square)
            nc.scalar.activation(t_sq[:], a_im[:], AF.Square)
            nc.vector.tensor_tensor(one_a2[:], one_a2[:], t_sq[:], OP.add)
            nc.vector.tensor_scalar_add(one_a2[:], one_a2[:], 1.0)

            # ---------- G = 1/(1 - A*WL^m), layout (128p=[p|p], 4096f) ----------
            ctab = gpool.tile([128, 4096], f32, tag="g_ctab")
            stab = gpool.tile([128, 4096], f32, tag="g_stab")
            nc.sync.dma_start(ctab[:], tbl_d["cm"][:])
            nc.sync.dma_start(stab[:], tbl_d["sm"][:])
            gq = gpool.tile([128, 4096], f32, tag="g_q")
            gt = gpool.tile([128, 4096], f32, tag="g_t")
            gdr = gpool.tile([128, 4096], f32, tag="g_dr")
            gn2 = gpool.tile([128, 4096], f32, tag="g_n2")
            g_r = gpool.tile([128, 4096], f32, tag="g_r")
            g_i = gpool.tile([128, 4096], f32, tag="g_i")
            # q = A_re*C + A_im*S
            nc.vector.tensor_scalar_mul(gq[:], ctab[:], a_re[:])
            nc.scalar.activation(gt[:], stab[:], AF.Identity, scale=a_im[:])
            nc.vector.tensor_tensor(gq[:], gq[:], gt[:], OP.add)
            # dr = 1 - q ; n2 = 1+|A|^2 - 2q ; rn = 1/n2
            nc.scalar.activation(gdr[:], gq[:], AF.Identity, scale=-1.0, bias=1.0)
            nc.vector.tensor_scalar(gn2[:], gq[:], -2.0, one_a2[:], OP.mult, OP.add)
            nc.vector.reciprocal(gn2[:], gn2[:])
            # di_n = A_im*C - A_re*S  (numerator of +Gi)
            g_t4 = gpool.tile([128, 4096], f32, tag="g_t4")
            nc.scalar.activation(gt[:], ctab[:], AF.Identity, scale=a_im[:])
            nc.scalar.activation(g_t4[:], stab[:], AF.Identity, scale=a_re[:])
            nc.vector.tensor_tensor(gt[:], gt[:], g_t4[:], OP.subtract)
            nc.vector.tensor_tensor(mdt(g_r[:]), gdr[:], gn2[:], OP.mult)
            nc.vector.tensor_tensor(mdt(g_i[:]), gt[:], gn2[:], OP.mult)

            # hi halves to base-partition-0 tiles (matmul rhs must match lhsT base)
            g_r_hi = gpool.tile([64, 4096], f32, tag="g_r_hi")
            g_i_hi = gpool.tile([64, 4096], f32, tag="g_i_hi")
            nc.sync.dma_start(mdt(g_r_hi[:]), mdt(g_r[64:128, :]))
            nc.sync.dma_start(mdt(g_i_hi[:]), mdt(g_i[64:128, :]))

            # ---------- Ks rows = BC @ G -> DRAM scratch ----------
            for j in range(16):  # m chunks of 512
                half = j // 8
                foff = (j % 8) * 512
                gr_sl = (g_r if half == 0 else g_r_hi)[0:64, foff:foff + 512]
                gi_sl = (g_i if half == 0 else g_i_hi)[0:64, foff:foff + 512]
                kr = pskp.tile([HSH, 512], f32, tag="ks_ps")
                ki = pskp.tile([HSH, 512], f32, tag="ks_ps")
                nc.tensor.matmul(kr[:], mdt(bct_r[:]), mdt(gr_sl), start=True, stop=False)
                nc.tensor.matmul(kr[:], mdt(bct_in[:]), mdt(gi_sl), start=False, stop=True)
                nc.tensor.matmul(ki[:], mdt(bct_i[:]), mdt(gr_sl), start=True, stop=False)
                nc.tensor.matmul(ki[:], mdt(bct_r[:]), mdt(gi_sl), start=False, stop=True)
                krs = ksp.tile([HSH, 512], f32, tag="ks_sb")
                kis = ksp.tile([HSH, 512], f32, tag="ks_sb")
                nc.scalar.activation(krs[:], kr[:], AF.Copy)
                nc.scalar.activation(kis[:], ki[:], AF.Copy)
                nc.sync.dma_start(ksr_d[:, j * 512:(j + 1) * 512], krs[:])
                nc.sync.dma_start(ksi_d[:, j * 512:(j + 1) * 512], kis[:])

            prologue_pools[1].__exit__(None, None, None)
            prologue_pools[0].__exit__(None, None, None)
            main_pools = (
                tc.tile_pool(name="io", bufs=IOBUFS),
                tc.tile_pool(name="mid", bufs=MIDBUFS),
                tc.tile_pool(name="ps", bufs=2, space=bass.MemorySpace.PSUM),
            )
            iop = main_pools[0].__enter__()
            midp = main_pools[1].__enter__()
            psp = main_pools[2].__enter__()
            # ---------- main loop: two interleaved h-lanes ----------
            SC = 1.0 / L

            def do_group(h, g, lane, kb):
                ksr_b, ksi_b, tr_b, ti_b, tir_b, tii_b = kb
                sfx = str(lane)
                bsl = slice(g * GB, (g + 1) * GB)
                u_t = iop.tile([64, GB, 128], f32, tag="u_t" + sfx, name="u_t")
                nc.sync.dma_start(
                    mdt(u_t[:]),
                    mdt(u_d[bsl, h, :].rearrange("b (n2 n1) -> n2 b n1", n1=128)))
                u_flat = u_t[:].rearrange("p b f -> p (b f)")

                # fwd stage 1
                y1r = psp.tile([64, 512], f32, tag="psA" + sfx, name="y1r")
                y1i = psp.tile([64, 512], f32, tag="psA" + sfx, name="y1i")
                nc.tensor.matmul(y1r[:], mdt(tb["f64r"][:]), mdt(u_flat))
                nc.tensor.matmul(y1i[:], mdt(tb["f64i"][:]), mdt(u_flat))

                # twiddle
                y1r_s = midp.tile([64, GB, 128], f32, tag="y1r_s" + sfx, name="y1r_s")
                y1i_s = midp.tile([64, GB, 128], f32, tag="y1i_s" + sfx, name="y1i_s")
                nc.scalar.activation(y1r_s[:].rearrange("p b f -> p (b f)"), y1r[:], AF.Copy)
                nc.scalar.activation(y1i_s[:].rearrange("p b f -> p (b f)"), y1i[:], AF.Copy)
                y2r = midp.tile([64, GB, 128], f32, tag="y2r" + sfx, name="y2r")
                y2i = midp.tile([64, GB, 128], f32, tag="y2i" + sfx, name="y2i")
                tw1 = midp.tile([64, GB, 128], f32, tag="tw1" + sfx, name="tw1")
                tw2 = midp.tile([64, GB, 128], f32, tag="tw2" + sfx, name="tw2")
                nc.vector.tensor_tensor(y2r[:], y1r_s[:], tr_b, OP.mult)
                nc.gpsimd.tensor_tensor(tw1[:], y1i_s[:], ti_b, OP.mult)
                nc.vector.tensor_tensor(y2r[:], y2r[:], tw1[:], OP.subtract)
                nc.vector.tensor_tensor(y2i[:], y1r_s[:], ti_b, OP.mult)
                nc.vector.tensor_tensor(tw2[:], y1i_s[:], tr_b, OP.mult)
                nc.gpsimd.tensor_tensor(y2i[:], y2i[:], tw2[:], OP.add)

                # fwd transposes
                y2t_ps = psp.tile([128, 512], f32, tag="psB" + sfx, name="y2t_ps")
                for j2 in range(GB):
                    nc.tensor.transpose(y2t_ps[:, j2 * 64:(j2 + 1) * 64],
                                        y2r[:, j2, :], tb["i64"][:])
                    nc.tensor.transpose(y2t_ps[:, 256 + j2 * 64:256 + (j2 + 1) * 64],
                                        y2i[:, j2, :], tb["i64"][:])
                y2t = midp.tile([128, 512], f32, tag="y2t" + sfx, name="y2t")
                nc.scalar.activation(mdt(y2t[:]), y2t_ps[:], AF.Copy)
                y2tr, y2ti = y2t[:, 0:256], y2t[:, 256:512]

                # fwd stage 2
                x_ps = psp.tile([128, 512], f32, tag="psB" + sfx, name="x_ps")
                xr, xi = x_ps[:, 0:256], x_ps[:, 256:512]
                nc.tensor.matmul(xr, mdt(tb["f128r"][:]), mdt(y2tr), start=True, stop=False)
                nc.tensor.matmul(xr, mdt(tb["f128in"][:]), mdt(y2ti), start=False, stop=True)
                nc.tensor.matmul(xi, mdt(tb["f128i"][:]), mdt(y2tr), start=True, stop=False)
                nc.tensor.matmul(xi, mdt(tb["f128r"][:]), mdt(y2ti), start=False, stop=True)

                # spectral
                xr_s = midp.tile([128, GB, 64], f32, tag="xr_s" + sfx, name="xr_s")
                xi_s = midp.tile([128, GB, 64], f32, tag="xi_s" + sfx, name="xi_s")
                nc.scalar.activation(xr_s[:].rearrange("p b f -> p (b f)"), xr, AF.Copy)
                nc.scalar.activation(xi_s[:].rearrange("p b f -> p (b f)"), xi, AF.Copy)
                s_sb = midp.tile([128, 2, GB, 64], f32, tag="s_sb" + sfx, name="s_sb")
                sr, si = s_sb[:, 0], s_sb[:, 1]
                sw1 = midp.tile([128, GB, 64], f32, tag="sw1" + sfx, name="sw1")
                sw2 = midp.tile([128, GB, 64], f32, tag="sw2" + sfx, name="sw2")
                nc.vector.tensor_tensor(mdt(sr[:]), xr_s[:], ksr_b, OP.mult)
                nc.gpsimd.tensor_tensor(sw1[:], xi_s[:], ksi_b, OP.mult)
                nc.vector.tensor_tensor(mdt(sr[:]), sr[:], sw1[:], OP.subtract)
                nc.vector.tensor_tensor(mdt(si[:]), xr_s[:], ksi_b, OP.mult)
                nc.vector.tensor_tensor(sw2[:], xi_s[:], ksr_b, OP.mult)
                nc.gpsimd.tensor_tensor(mdt(si[:]), si[:], sw2[:], OP.add)
                sr_f = sr.rearrange("p b f -> p (b f)")
                si_f = si.rearrange("p b f -> p (b f)")

                # inv stage 1
                z1_ps = psp.tile([128, 512], f32, tag="psB" + sfx, name="z1_ps")
                z1r, z1i = z1_ps[:, 0:256], z1_ps[:, 256:512]
                nc.tensor.matmul(z1r, mdt(tb["f128r"][:]), mdt(sr_f), start=True, stop=False)
                nc.tensor.matmul(z1r, mdt(tb["f128i"][:]), mdt(si_f), start=False, stop=True)
                nc.tensor.matmul(z1i, mdt(tb["f128r"][:]), mdt(si_f), start=True, stop=False)
                nc.tensor.matmul(z1i, mdt(tb["f128in"][:]), mdt(sr_f), start=False, stop=True)

                # inv twiddle
                z1r_s = midp.tile([128, GB, 64], f32, tag="z1r_s" + sfx, name="z1r_s")
                z1i_s = midp.tile([128, GB, 64], f32, tag="z1i_s" + sfx, name="z1i_s")
                nc.scalar.activation(z1r_s[:].rearrange("p b f -> p (b f)"), z1r, AF.Copy)
                nc.scalar.activation(z1i_s[:].rearrange("p b f -> p (b f)"), z1i, AF.Copy)
                z2r = midp.tile([128, GB, 64], f32, tag="z2r" + sfx, name="z2r")
                z2i = midp.tile([128, GB, 64], f32, tag="z2i" + sfx, name="z2i")
                zw1 = midp.tile([128, GB, 64], f32, tag="zw1" + sfx, name="zw1")
                zw2 = midp.tile([128, GB, 64], f32, tag="zw2" + sfx, name="zw2")
                nc.vector.tensor_tensor(z2r[:], z1r_s[:], tir_b, OP.mult)
                nc.gpsimd.tensor_tensor(zw1[:], z1i_s[:], tii_b, OP.mult)
                nc.vector.tensor_tensor(z2r[:], z2r[:], zw1[:], OP.subtract)
                nc.vector.tensor_tensor(z2i[:], z1r_s[:], tii_b, OP.mult)
                nc.vector.tensor_tensor(zw2[:], z1i_s[:], tir_b, OP.mult)
                nc.gpsimd.tensor_tensor(z2i[:], z2i[:], zw2[:], OP.add)

                # inv transposes
                z2tr_ps = psp.tile([64, 512], f32, tag="psA" + sfx, name="z2tr_ps")
                z2ti_ps = psp.tile([64, 512], f32, tag="psA" + sfx, name="z2ti_ps")
                for j2 in range(GB):
                    nc.tensor.transpose(z2tr_ps[:, j2 * 128:(j2 + 1) * 128],
                                        z2r[:, j2, :], tb["i128"][:])
                    nc.tensor.transpose(z2ti_ps[:, j2 * 128:(j2 + 1) * 128],
                                        z2i[:, j2, :], tb["i128"][:])
                z2t = midp.tile([64, 1024], f32, tag="z2t" + sfx, name="z2t")
                nc.scalar.activation(mdt(z2t[:, 0:512]), z2tr_ps[:], AF.Copy)
                nc.scalar.activation(mdt(z2t[:, 512:1024]), z2ti_ps[:], AF.Copy)

                # inv stage 2 (real part)
                xo_ps = psp.tile([64, 512], f32, tag="psB" + sfx, name="xo_ps")
                nc.tensor.matmul(xo_ps[:], mdt(tb["f64r"][:]), mdt(z2t[:, 0:512]),
                                 start=True, stop=False)
                nc.tensor.matmul(xo_ps[:], mdt(tb["f64i"][:]), mdt(z2t[:, 512:1024]),
                                 start=False, stop=True)

                # final
                ud = midp.tile([64, 512], f32, tag="ud" + sfx, name="ud")
                nc.gpsimd.tensor_scalar_mul(ud[:], u_flat, d_b[:, h:h + 1])
                yt = midp.tile([64, 512], f32, tag="yt" + sfx, name="yt")
                nc.vector.scalar_tensor_tensor(yt[:], xo_ps[:], SC, ud[:],
                                               OP.mult, OP.add)
                yo = iop.tile([64, GB, 128], f32, tag="yo" + sfx, name="yo")
                nc.scalar.activation(yo[:].rearrange("p b f -> p (b f)"), yt[:], AF.Tanh)
                nc.sync.dma_start(
                    y_d[bsl, h, :].rearrange("b (n2 n1) -> n2 b n1", n1=128), yo[:])

            def prep_h(h, lane):
                sfx = str(lane)
                ksr_t = ksp.tile([128, 64], f32, tag="ks_h" + sfx, name="ksr_t")
                ksi_t = ksp.tile([128, 64], f32, tag="ks_h" + sfx, name="ksi_t")
                nc.sync.dma_start(ksr_t[:], ksr_d[h].rearrange("(k1 k2) -> k1 k2", k2=64))
                nc.sync.dma_start(ksi_t[:], ksi_d[h].rearrange("(k1 k2) -> k1 k2", k2=64))
                return (
                    ksr_t[:].unsqueeze(1).broadcast_to([128, GB, 64]),
                    ksi_t[:].unsqueeze(1).broadcast_to([128, GB, 64]),
                    tb["tr"][:].unsqueeze(1).broadcast_to([64, GB, 128]),
                    tb["ti"][:].unsqueeze(1).broadcast_to([64, GB, 128]),
                    tb["tir"][:].unsqueeze(1).broadcast_to([128, GB, 64]),
                    tb["tii"][:].unsqueeze(1).broadcast_to([128, GB, 64]),
                )

            for _rep in range(REPEAT):
                for hp in range(HSH // 2):
                    hA, hB = 2 * hp, 2 * hp + 1
                    kbA = prep_h(hA, 0)
                    kbB = prep_h(hB, 1)
                    for g in range(NG):
                        do_group(hA, g, 0, kbA)
                        do_group(hB, g, 1, kbB)
            for mp in reversed(main_pools):
                mp.__exit__(None, None, None)

    nc.compile()
    return nc


def _get_program():
    key = ("prog", F32R, REPEAT, MIDBUFS, IOBUFS)
    if key not in _CACHE:
        import concourse.bass as bass
        import concourse.tile as tile
        from concourse import mybir, bacc
        _CACHE[key] = _build((bass, tile, mybir, bacc))
    return _CACHE[key]


def kernel(u, A_re, A_im, BC_re, BC_im, D):
    from concourse.bass_utils import run_bass_kernel_spmd

    u = np.ascontiguousarray(u, dtype=np.float32)
    tabs = _tables()
    nc = _get_program()

    in_maps = []
    for c in range(NCORES):
        hs = slice(c * HSH, (c + 1) * HSH)
        m = {
            "u_sh": np.ascontiguousarray(u[:, hs, :]),
            "a_re": np.ascontiguousarray(
                np.concatenate([A_re, A_re]).reshape(2 * P, 1).astype(np.float32)),
            "a_im": np.ascontiguousarray(
                np.concatenate([A_im, A_im]).reshape(2 * P, 1).astype(np.float32)),
            "bct_r": np.ascontiguousarray(BC_re[hs].T.astype(np.float32)),
            "bct_i": np.ascontiguousarray(BC_im[hs].T.astype(np.float32)),
            "bct_i_neg": np.ascontiguousarray(-BC_im[hs].T.astype(np.float32)),
            "d_sh": np.ascontiguousarray(D[hs].reshape(1, HSH).astype(np.float32)),
        }
        m.update(tabs)
        in_maps.append(m)

    res = None
    last_err = None
    for attempt in range(3):
        try:
            res = run_bass_kernel_spmd(nc, in_maps, list(range(NCORES)))
            break
        except Exception as e:  # transient NRT_EXEC_UNIT_UNRECOVERABLE flakes
            last_err = e
            import time as _time
            _time.sleep(2.0)
    if res is None:
        raise last_err
    out = np.concatenate([res.results[c]["y_sh"] for c in range(NCORES)], axis=1)
    return out.astype(np.float32)


if __name__ == "__main__":
    rng = np.random.default_rng(0)
    u = rng.standard_normal((B, H, L), dtype=np.float32)
    A_re = rng.uniform(0.5, 0.99, P).astype(np.float32)
    A_im = rng.uniform(-0.5, 0.5, P).astype(np.float32)
    BC_re = rng.standard_normal((H, P), dtype=np.float32)
    BC_im = rng.standard_normal((H, P), dtype=np.float32)
    D = rng.uniform(0, 1, H).astype(np.float32)
    y = kernel(u=u, A_re=A_re, A_im=A_im, BC_re=BC_re, BC_im=BC_im, D=D)
    print("out", y.shape, y.dtype)



# revision 6
# speedup vs baseline: 4.1355x; 4.1355x over previous
"""Trainium2 Bass kernel for nn_FFTConv: y = tanh(Re(ifft(fft(u)*Ks)) + D*u).

v2 redesign:
  * Complex packing: conv with a REAL kernel commutes with Re/Im, so pack
    z[j] = u[j] + i*u[j+8] (j in [0,8)) per h -> halves all work. The real
    kernel's spectrum comes from 128 poles {A, conj(A)} with coefficients
    {BC/2, conj(BC)/2}; D*u and the 1/L ifft scale fold into the spectrum
    (delta kernel: +D to every frequency bin).
  * All matmuls bf16 (1 cyc/row), all elementwise in bf16 SBUF (DVE 2x mode),
    full 128-partition layouts via block-diagonal stationaries for the
    64-point DFT stages.
  * H-sharded across 8 cores (32 ch/core); per h: 8 packed complex rows,
    free dim 512 everywhere; 2 software-pipelined lanes.

Layout per h (L = 8192 = 64*128, n = n1 + 128*n2, m = k2 + 64*k1):
  u: [128p=(g,n2), c=2(re/im), bb=4, n1=128]   (g*4+bb = packed row)
  stage1  (PE): BD(F64) over n2       -> S1  [p=(g,k2), bb, n1]
  twiddle (DVE): * W_L^{k2 n1}
  transp  (PE): per (c,bb) 128x128    -> TP  [p=n1, c, bb, (g,k2)]
  stage2  (PE): F128 over n1          -> S2  [p=k1, bb, g, k2]
  spectral(DVE): * Ks[m]/L (+D)
  inv1    (PE): conj(F128) over k1    -> Z1  [p=o2, bb, g, k2]
  invtw   (DVE/Pool): * W_L^{+o2 k2}
  transp  (PE)                        -> TQ  [p=(g,k2), c, bb, o2]
  final   (PE): BD(conj(F64)) over k2 -> F   [p=(g,n2), bb, o2=n1]
  tanh    (Act, from PSUM)            -> y rows 0:8 = Re, 8:16 = Im
"""
import os
import sys
import numpy as np

for p in ("/opt/trn_rl_repo", "/root/.axon_site/_ro/trn_rl_repo"):
    if os.path.isdir(p) and p not in sys.path:
        sys.path.append(p)

import ml_dtypes

BF16 = ml_dtypes.bfloat16

B, H, L, P = 16, 256, 8192, 64
NCORES = 8
HSH = H // NCORES          # 32 channels per core
NLANES = int(os.environ.get("KERNEL_NLANES", "4"))
REPEAT = int(os.environ.get("KERNEL_REPEAT", "1"))
MIDBUFS = int(os.environ.get("KERNEL_MIDBUFS", "2"))
IOBUFS = int(os.environ.get("KERNEL_IOBUFS", "2"))
PFBUFS = int(os.environ.get("KERNEL_PFBUFS", "2"))
PTBUFS = int(os.environ.get("KERNEL_PTBUFS", "0"))
SKEW = int(os.environ.get("KERNEL_SKEW", "1"))

_CACHE = {}


def _tables():
    a64 = np.arange(64)
    a128 = np.arange(128)
    c64 = np.cos(2 * np.pi * np.outer(a64, a64) / 64)
    s64 = np.sin(2 * np.pi * np.outer(a64, a64) / 64)
    c128 = np.cos(2 * np.pi * np.outer(a128, a128) / 128)
    s128 = np.sin(2 * np.pi * np.outer(a128, a128) / 128)
    z64 = np.zeros((64, 64))

    def bd(x):
        return np.block([[x, z64], [z64, x]])

    k2v = a128 % 64
    thT = 2 * np.pi * np.outer(k2v, a128) / L        # [(g,k2), n1]
    thI = 2 * np.pi * np.outer(a128, a64) / L        # [o2, k2]
    t2r, t2i = np.cos(thT), -np.sin(thT)
    tir, tii = np.cos(thI), np.sin(thI)
    t = {
        "bd64r": bd(c64), "bd64i": bd(-s64), "bd64in": bd(s64),
        "f128r": c128, "f128i": -s128, "f128in": s128,
        "i128": np.eye(128),
        # paired twiddle tables: [re|im] and [im|re] side by side
        "t2a": np.concatenate([t2r, t2i], axis=1),   # [128, 256]
        "t2b": np.concatenate([t2i, t2r], axis=1),
        "tia": np.concatenate([tir, tii], axis=1),   # [128, 128]
        "tib": np.concatenate([tii, tir], axis=1),
    }
    t = {k: v.astype(BF16) for k, v in t.items()}
    m = np.arange(L)
    t["cm"] = np.cos(2 * np.pi * m / L).reshape(1, L).astype(np.float32)
    t["sm"] = np.sin(2 * np.pi * m / L).reshape(1, L).astype(np.float32)
    return t


TBL_SHAPES = {
    "bd64r": [128, 128], "bd64i": [128, 128], "bd64in": [128, 128],
    "f128r": [128, 128], "f128i": [128, 128], "f128in": [128, 128],
    "i128": [128, 128], "t2a": [128, 256], "t2b": [128, 256],
    "tia": [128, 128], "tib": [128, 128],
    "cm": [1, L], "sm": [1, L],
}


def _build(nc_mod):
    bass, tile, mybir, bacc = nc_mod
    dt = mybir.dt
    f32 = dt.float32
    bf16 = dt.bfloat16
    R = dt.float32r

    def fr(ap):
        return ap.bitcast(R)

    nc = bacc.Bacc("TRN2", target_bir_lowering=False, debug=False)
    AF = mybir.ActivationFunctionType
    OP = mybir.AluOpType
    SC = 1.0 / L

    # ---------------- DRAM parameters ----------------
    # u2/y2 are host-relayouted: [p=(g,n2), h, (bb c), n1] so each h is one
    # full-width contiguous DMA (2KB/partition in, 4KB/partition out).
    u_d = nc.declare_dram_parameter("u2_sh", [128, HSH, 8, 128], bf16, isOutput=False)
    y_d = nc.declare_dram_parameter("y2_sh", [128, HSH, 8, 128], f32, isOutput=True)
    ar_d = nc.declare_dram_parameter("a_re2", [128, 1], f32, isOutput=False)
    ai_d = nc.declare_dram_parameter("a_im2", [128, 1], f32, isOutput=False)
    a2_d = nc.declare_dram_parameter("a2one", [128, 1], f32, isOutput=False)
    bcr_d = nc.declare_dram_parameter("bct_r", [128, HSH], f32, isOutput=False)
    bci_d = nc.declare_dram_parameter("bct_i", [128, HSH], f32, isOutput=False)
    bcin_d = nc.declare_dram_parameter("bct_in", [128, HSH], f32, isOutput=False)
    dl_d = nc.declare_dram_parameter("d_l", [HSH, 1], f32, isOutput=False)
    tbl_d = {}
    for n, shp in TBL_SHAPES.items():
        dty = f32 if n in ("cm", "sm") else bf16
        tbl_d[n] = nc.declare_dram_parameter(n, shp, dty, isOutput=False)

    ks_d = nc.dram_tensor("ks_scratch", [HSH, 2, L], bf16)

    with tile.TileContext(nc) as tc:
        with tc.tile_pool(name="const", bufs=1) as cpool:
            tb = {}
            for n in TBL_SHAPES:
                if n in ("cm", "sm"):
                    continue
                tb[n] = cpool.tile(TBL_SHAPES[n], bf16, tag=f"c_{n}", name=f"c_{n}")
                nc.sync.dma_start(tb[n][:], tbl_d[n][:])
            a_re = cpool.tile([128, 1], f32, tag="a_re")
            a_im = cpool.tile([128, 1], f32, tag="a_im")
            a2one = cpool.tile([128, 1], f32, tag="a2one")
            d_l = cpool.tile([HSH, 1], f32, tag="d_l")
            nc.sync.dma_start(a_re[:], ar_d[:])
            nc.sync.dma_start(a_im[:], ai_d[:])
            nc.sync.dma_start(a2one[:], a2_d[:])
            nc.sync.dma_start(d_l[:], dl_d[:])
            bct_r = cpool.tile([128, HSH], f32, tag="bct_r")
            bct_i = cpool.tile([128, HSH], f32, tag="bct_i")
            bct_in = cpool.tile([128, HSH], f32, tag="bct_in")
            nc.sync.dma_start(fr(bct_r[:]), fr(bcr_d[:]))
            nc.sync.dma_start(fr(bct_i[:]), fr(bci_d[:]))
            nc.sync.dma_start(fr(bct_in[:]), fr(bcin_d[:]))

            # ---------- prologue: G = 1/(1 - A'*W^m) for 128 poles ----------
            prologue_pools = (
                tc.tile_pool(name="gwork", bufs=1),
                tc.tile_pool(name="ksb", bufs=3),
                tc.tile_pool(name="psk", bufs=2, space=bass.MemorySpace.PSUM),
            )
            gpool = prologue_pools[0].__enter__()
            ksbp = prologue_pools[1].__enter__()
            pskp = prologue_pools[2].__enter__()

            for c in range(2):
                msl = slice(c * 4096, (c + 1) * 4096)
                ctab = gpool.tile([128, 4096], f32, tag="ctab")
                stab = gpool.tile([128, 4096], f32, tag="stab")
                nc.sync.dma_start(ctab[:], tbl_d["cm"][:, msl].broadcast_to([128, 4096]))
                nc.sync.dma_start(stab[:], tbl_d["sm"][:, msl].broadcast_to([128, 4096]))
                t1 = gpool.tile([128, 4096], f32, tag="t1")
                t2 = gpool.tile([128, 4096], f32, tag="t2")
                nc.scalar.activation(t1[:], ctab[:], AF.Identity, scale=a_re[:])
                nc.scalar.activation(t2[:], stab[:], AF.Identity, scale=a_im[:])
                q = gpool.tile([128, 4096], f32, tag="q")
                nc.gpsimd.tensor_tensor(q[:], t1[:], t2[:], OP.add)
                rn = gpool.tile([128, 4096], f32, tag="rn")
                nc.scalar.activation(rn[:], q[:], AF.Identity, scale=-2.0, bias=a2one[:])
                nc.vector.reciprocal(rn[:], rn[:])
                dr = gpool.tile([128, 4096], f32, tag="dr")
                nc.scalar.activation(dr[:], q[:], AF.Identity, scale=-1.0, bias=1.0)
                t1b = gpool.tile([128, 4096], f32, tag="t1")
                t2b = gpool.tile([128, 4096], f32, tag="t2")
                nc.scalar.activation(t1b[:], ctab[:], AF.Identity, scale=a_im[:])
                nc.scalar.activation(t2b[:], stab[:], AF.Identity, scale=a_re[:])
                di = gpool.tile([128, 4096], f32, tag="di")
                nc.gpsimd.tensor_tensor(di[:], t1b[:], t2b[:], OP.subtract)
                gr = gpool.tile([128, 4096], f32, tag="gr")
                gi = gpool.tile([128, 4096], f32, tag="gi")
                nc.gpsimd.tensor_tensor(fr(gr[:]), dr[:], rn[:], OP.mult)
                nc.gpsimd.tensor_tensor(fr(gi[:]), di[:], rn[:], OP.mult)

                for j in range(8):
                    fsl = slice(j * 512, (j + 1) * 512)
                    m0 = c * 4096 + j * 512
                    kr = pskp.tile([HSH, 512], f32, tag="kr")
                    ki = pskp.tile([HSH, 512], f32, tag="ki")
                    nc.tensor.matmul(kr[:], fr(bct_r[:]), fr(gr[:, fsl]), start=True, stop=False)
                    nc.tensor.matmul(kr[:], fr(bct_in[:]), fr(gi[:, fsl]), start=False, stop=True)
                    nc.tensor.matmul(ki[:], fr(bct_i[:]), fr(gr[:, fsl]), start=True, stop=False)
                    nc.tensor.matmul(ki[:], fr(bct_r[:]), fr(gi[:, fsl]), start=False, stop=True)
                    krs = ksbp.tile([HSH, 512], bf16, tag="krs")
                    kis = ksbp.tile([HSH, 512], bf16, tag="kis")
                    # kr/L + D/L  (folds D*u skip and ifft 1/L into the spectrum)
                    nc.scalar.activation(krs[:], kr[:], AF.Identity, scale=SC, bias=d_l[:])
                    nc.vector.tensor_scalar_mul(kis[:], ki[:], SC)
                    nc.sync.dma_start(ks_d[:, 0, m0:m0 + 512], krs[:])
                    nc.sync.dma_start(ks_d[:, 1, m0:m0 + 512], kis[:])

            prologue_pools[2].__exit__(None, None, None)
            prologue_pools[1].__exit__(None, None, None)
            prologue_pools[0].__exit__(None, None, None)

            # preload the whole spectrum [k1, h, (r,i,i,r), k2] via 4 bulk DMAs
            ks_all = cpool.tile([128, HSH, 4, 64], bf16, tag="ks_all")
            for cc, src in enumerate((0, 1, 1, 0)):
                nc.sync.dma_start(
                    ks_all[:, :, cc, :],
                    ks_d[:, src, :].rearrange("h (k1 k2) -> k1 h k2", k2=64))

            # ---------- main loop ----------
            main_pools = [
                tc.tile_pool(name="io", bufs=IOBUFS),
                tc.tile_pool(name="mid", bufs=MIDBUFS),
                tc.tile_pool(name="pf", bufs=PFBUFS, space=bass.MemorySpace.PSUM),
            ]
            if PTBUFS > 0:
                main_pools.append(
                    tc.tile_pool(name="pt", bufs=PTBUFS, space=bass.MemorySpace.PSUM))
            iop = main_pools[0].__enter__()
            midp = main_pools[1].__enter__()
            pfp = main_pools[2].__enter__()
            ptp = main_pools[3].__enter__() if PTBUFS > 0 else None

            t2a_b = tb["t2a"][:].unsqueeze(1).broadcast_to([128, 4, 256])
            t2b_b = tb["t2b"][:].unsqueeze(1).broadcast_to([128, 4, 256])
            tia_b = tb["tia"][:].unsqueeze(1).broadcast_to([128, 8, 128])
            tib_b = tb["tib"][:].unsqueeze(1).broadcast_to([128, 8, 128])

            def stages(h, lane):
                sfx = str(lane)

                def mtile(tag, w=512):
                    return midp.tile([128, w], bf16, tag=tag + sfx, name=tag)

                # --- loads ---
                ks_a = ks_all[:, h, 0:2, :].rearrange("p a b -> p (a b)").unsqueeze(
                    1).broadcast_to([128, 8, 128])
                ks_b = ks_all[:, h, 2:4, :].rearrange("p a b -> p (a b)").unsqueeze(
                    1).broadcast_to([128, 8, 128])
                # packing: complex row j = g*4+bb is u[2j] + i*u[2j+1];
                # SBUF layout [p, bb, c, n1]; host pre-layouts u2 so one
                # full-width contiguous DMA loads the whole h.
                uc = iop.tile([128, 2, 512], bf16, tag="uc" + sfx, name="uc")
                nc.sync.dma_start(
                    uc[:].rearrange("p c (bb n1) -> p (c bb) n1", n1=128), u_d[:, h])
                yield

                # --- stage 1: BD(F64) over n2 ---
                S1i = pfp.tile([128, 512], f32, tag="pf" + sfx, name="S1i")
                S1r = pfp.tile([128, 512], f32, tag="pf" + sfx, name="S1r")
                ure, uim = uc[:, 0], uc[:, 1]
                nc.tensor.matmul(S1i[:], tb["bd64i"][:], ure, start=True, stop=False)
                nc.tensor.matmul(S1i[:], tb["bd64r"][:], uim, start=False, stop=True)
                nc.tensor.matmul(S1r[:], tb["bd64r"][:], ure, start=True, stop=False)
                nc.tensor.matmul(S1r[:], tb["bd64in"][:], uim, start=False, stop=True)
                # y1 layout [p, bb, (c n1)]
                y1 = midp.tile([128, 4, 256], bf16, tag="y1" + sfx, name="y1")
                nc.scalar.activation(
                    y1[:, :, 0:128], S1r[:].rearrange("p (a b) -> p a b", a=4), AF.Copy)
                nc.scalar.activation(
                    y1[:, :, 128:256], S1i[:].rearrange("p (a b) -> p a b", a=4), AF.Copy)
                yield

                # --- fwd twiddle: 2 paired mults (DVE) + 2 addsubs ---
                ma = midp.tile([128, 4, 256], bf16, tag="mA" + sfx, name="ma")
                mb = midp.tile([128, 4, 256], bf16, tag="mB" + sfx, name="mb")
                nc.vector.tensor_tensor(ma[:], y1[:], t2a_b, OP.mult)
                nc.vector.tensor_tensor(mb[:], y1[:], t2b_b, OP.mult)
                y2r = ma[:, :, 0:128]
                y2i = mb[:, :, 0:128]
                nc.gpsimd.tensor_tensor(y2r, ma[:, :, 0:128], ma[:, :, 128:256], OP.subtract)
                nc.gpsimd.tensor_tensor(y2i, mb[:, :, 0:128], mb[:, :, 128:256], OP.add)
                yield

                # --- fwd transposes ---
                TP = ptp.tile([128, 2, 512], bf16, tag="pt" + sfx, name="TP") \
                    if ptp is not None else \
                    pfp.tile([128, 2, 512], bf16, tag="pf" + sfx, name="TP")
                for cc, srcv in ((0, y2r), (1, y2i)):
                    tpv = TP[:, cc].rearrange("p (a b) -> p a b", a=4)
                    for bb in range(4):
                        nc.tensor.transpose(tpv[:, bb, :], srcv[:, bb, :], tb["i128"][:])
                y2t = midp.tile([128, 2, 512], bf16, tag="y2t" + sfx, name="y2t")
                nc.vector.tensor_scalar_add(
                    y2t[:].rearrange("p a b -> p (a b)"),
                    TP[:].rearrange("p a b -> p (a b)"), 0.0)
                yield

                # --- stage 2: F128 over n1 ---
                S2i = pfp.tile([128, 512], f32, tag="pf" + sfx, name="S2i")
                S2r = pfp.tile([128, 512], f32, tag="pf" + sfx, name="S2r")
                y2tr, y2ti = y2t[:, 0], y2t[:, 1]
                nc.tensor.matmul(S2i[:], tb["f128i"][:], y2tr, start=True, stop=False)
                nc.tensor.matmul(S2i[:], tb["f128r"][:], y2ti, start=False, stop=True)
                nc.tensor.matmul(S2r[:], tb["f128r"][:], y2tr, start=True, stop=False)
                nc.tensor.matmul(S2r[:], tb["f128in"][:], y2ti, start=False, stop=True)
                # x layout [p, (bb g), (c k2)]
                x = midp.tile([128, 8, 128], bf16, tag="x" + sfx, name="x")
                nc.scalar.activation(
                    x[:, :, 0:64], S2r[:].rearrange("p (a b) -> p a b", a=8), AF.Copy)
                nc.scalar.activation(
                    x[:, :, 64:128], S2i[:].rearrange("p (a b) -> p a b", a=8), AF.Copy)
                yield

                # --- spectral multiply: 2 paired mults (DVE) + 2 addsubs (Pool) ---
                ma = midp.tile([128, 8, 128], bf16, tag="mA" + sfx, name="ma")
                mb = midp.tile([128, 8, 128], bf16, tag="mB" + sfx, name="mb")
                nc.vector.tensor_tensor(ma[:], x[:], ks_a, OP.mult)
                nc.vector.tensor_tensor(mb[:], x[:], ks_b, OP.mult)
                sre = midp.tile([128, 512], bf16, tag="w1" + sfx, name="sre")
                sim_ = midp.tile([128, 512], bf16, tag="w2" + sfx, name="sim")
                s_re, s_im = sre[:], sim_[:]
                nc.gpsimd.tensor_tensor(
                    sre[:].rearrange("p (a b) -> p a b", a=8),
                    ma[:, :, 0:64], ma[:, :, 64:128], OP.subtract)
                nc.gpsimd.tensor_tensor(
                    sim_[:].rearrange("p (a b) -> p a b", a=8),
                    mb[:, :, 0:64], mb[:, :, 64:128], OP.add)
                yield

                # --- inverse stage 1: conj(F128) over k1 ---
                Z1i = pfp.tile([128, 512], f32, tag="pf" + sfx, name="Z1i")
                Z1r = pfp.tile([128, 512], f32, tag="pf" + sfx, name="Z1r")
                nc.tensor.matmul(Z1i[:], tb["f128in"][:], s_re, start=True, stop=False)
                nc.tensor.matmul(Z1i[:], tb["f128r"][:], s_im, start=False, stop=True)
                nc.tensor.matmul(Z1r[:], tb["f128r"][:], s_re, start=True, stop=False)
                nc.tensor.matmul(Z1r[:], tb["f128i"][:], s_im, start=False, stop=True)
                # z1 layout [p, (bb g), (c k2)]
                z1 = midp.tile([128, 8, 128], bf16, tag="z1" + sfx, name="z1")
                nc.scalar.activation(
                    z1[:, :, 0:64], Z1r[:].rearrange("p (a b) -> p a b", a=8), AF.Copy)
                nc.scalar.activation(
                    z1[:, :, 64:128], Z1i[:].rearrange("p (a b) -> p a b", a=8), AF.Copy)
                yield

                # --- inverse twiddle: 2 paired mults (DVE) + 2 addsubs (Pool) ---
                ma = midp.tile([128, 8, 128], bf16, tag="mA" + sfx, name="ma")
                mb = midp.tile([128, 8, 128], bf16, tag="mB" + sfx, name="mb")
                nc.vector.tensor_tensor(ma[:], z1[:], tia_b, OP.mult)
                nc.vector.tensor_tensor(mb[:], z1[:], tib_b, OP.mult)
                z2r = midp.tile([128, 512], bf16, tag="w1" + sfx, name="z2r")
                z2i = midp.tile([128, 512], bf16, tag="w2" + sfx, name="z2i")
                nc.gpsimd.tensor_tensor(
                    z2r[:].rearrange("p (a b) -> p a b", a=8),
                    ma[:, :, 0:64], ma[:, :, 64:128], OP.subtract)
                nc.gpsimd.tensor_tensor(
                    z2i[:].rearrange("p (a b) -> p a b", a=8),
                    mb[:, :, 0:64], mb[:, :, 64:128], OP.add)
                yield

                # --- inverse transposes ---
                TQ = ptp.tile([128, 2, 512], bf16, tag="pt" + sfx, name="TQ") \
                    if ptp is not None else \
                    pfp.tile([128, 2, 512], bf16, tag="pf" + sfx, name="TQ")
                for cc, zsrc in ((0, z2r), (1, z2i)):
                    tqv = TQ[:, cc].rearrange("p (a b) -> p a b", a=4)
                    zsv = zsrc[:].rearrange("p (a b) -> p a b", a=4)
                    for bb in range(4):
                        nc.tensor.transpose(tqv[:, bb, :], zsv[:, bb, :], tb["i128"][:])
                z2t = midp.tile([128, 2, 512], bf16, tag="z2t" + sfx, name="z2t")
                nc.vector.tensor_scalar_add(
                    z2t[:].rearrange("p a b -> p (a b)"),
                    TQ[:].rearrange("p a b -> p (a b)"), 0.0)
                yield

                # --- final: BD(conj(F64)) over k2, tanh, store ---
                Fi = pfp.tile([128, 512], f32, tag="pf" + sfx, name="Fi")
                Fr = pfp.tile([128, 512], f32, tag="pf" + sfx, name="Fr")
                z2tr, z2ti = z2t[:, 0], z2t[:, 1]
                nc.tensor.matmul(Fi[:], tb["bd64in"][:], z2tr, start=True, stop=False)
                nc.tensor.matmul(Fi[:], tb["bd64r"][:], z2ti, start=False, stop=True)
                nc.tensor.matmul(Fr[:], tb["bd64r"][:], z2tr, start=True, stop=False)
                nc.tensor.matmul(Fr[:], tb["bd64i"][:], z2ti, start=False, stop=True)
                yo = iop.tile([128, 2, 512], f32, tag="yo" + sfx, name="yo")
                nc.scalar.activation(yo[:, 0], Fr[:], AF.Tanh)
                nc.scalar.activation(yo[:, 1], Fi[:], AF.Tanh)
                nc.sync.dma_start(
                    y_d[:, h], yo[:].rearrange("p c (bb n1) -> p (c bb) n1", n1=128))
                yield

            def lane_stream(ln):
                for _rep in range(REPEAT):
                    for h in range(ln, HSH, NLANES):
                        yield from stages(h, ln)

            gens = [lane_stream(ln) for ln in range(NLANES)]
            done = [False] * NLANES
            # prime lanes with a stage skew so engine queues interleave
            # different pipeline stages instead of running in lockstep
            for ln in range(NLANES):
                for _ in range((NLANES - 1 - ln) * SKEW):
                    try:
                        next(gens[ln])
                    except StopIteration:
                        done[ln] = True
                        break
            while not all(done):
                for gi_, g in enumerate(gens):
                    if not done[gi_]:
                        try:
                            next(g)
                        except StopIteration:
                            done[gi_] = True

            for mp in reversed(main_pools):
                mp.__exit__(None, None, None)

    nc.compile()
    return nc


def _get_program():
    key = ("prog", NLANES, REPEAT, MIDBUFS, IOBUFS, PFBUFS, PTBUFS)
    if key not in _CACHE:
        import concourse.bass as bass
        import concourse.tile as tile
        from concourse import mybir, bacc
        _CACHE[key] = _build((bass, tile, mybir, bacc))
    return _CACHE[key]


def _u_relayout(u_bf_core):
    """[16, HSH, L] -> [p=(g,n2), h, (c bb), n1]  (b = g*8 + bb*2 + c)."""
    us = u_bf_core.reshape(2, 4, 2, HSH, 64, 128)   # g bb c h n2 n1
    return np.ascontiguousarray(us.transpose(0, 4, 3, 2, 1, 5).reshape(128, HSH, 8, 128))


def _y_relayout(y2_core):
    """[p=(g,n2), h, (c bb), n1] -> [16, HSH, L]."""
    ys = y2_core.reshape(2, 64, HSH, 2, 4, 128)      # g n2 h c bb n1
    return ys.transpose(0, 4, 3, 2, 1, 5).reshape(16, HSH, L)


def make_in_maps(u, A_re, A_im, BC_re, BC_im, D):
    tabs = _tables()
    u_bf = np.ascontiguousarray(u).astype(BF16)
    a_re2 = np.concatenate([A_re, A_re]).reshape(128, 1).astype(np.float32)
    a_im2 = np.concatenate([A_im, -A_im]).reshape(128, 1).astype(np.float32)
    a2one = (1.0 + A_re.astype(np.float64) ** 2 + A_im.astype(np.float64) ** 2)
    a2one = np.concatenate([a2one, a2one]).reshape(128, 1).astype(np.float32)
    in_maps = []
    for c in range(NCORES):
        hs = slice(c * HSH, (c + 1) * HSH)
        bcr = BC_re[hs].T.astype(np.float32) / 2
        bci = BC_im[hs].T.astype(np.float32) / 2
        m = {
            "u2_sh": _u_relayout(u_bf[:, hs, :]),
            "a_re2": a_re2,
            "a_im2": a_im2,
            "a2one": a2one,
            "bct_r": np.ascontiguousarray(np.concatenate([bcr, bcr], axis=0)),
            "bct_i": np.ascontiguousarray(np.concatenate([bci, -bci], axis=0)),
            "bct_in": np.ascontiguousarray(np.concatenate([-bci, bci], axis=0)),
            "d_l": np.ascontiguousarray(
                (D[hs] / L).reshape(HSH, 1).astype(np.float32)),
        }
        m.update(tabs)
        in_maps.append(m)
    return in_maps


def kernel(u, A_re, A_im, BC_re, BC_im, D):
    from concourse.bass_utils import run_bass_kernel_spmd

    nc = _get_program()
    in_maps = make_in_maps(u, A_re, A_im, BC_re, BC_im, D)

    res = None
    last_err = None
    for attempt in range(3):
        try:
            res = run_bass_kernel_spmd(nc, in_maps, list(range(NCORES)))
            break
        except Exception as e:  # transient NRT_EXEC_UNIT_UNRECOVERABLE flakes
            last_err = e
            import time as _time
            _time.sleep(2.0)
    if res is None:
        raise last_err
    out = np.concatenate(
        [_y_relayout(res.results[c]["y2_sh"]) for c in range(NCORES)], axis=1)
    return np.ascontiguousarray(out, dtype=np.float32)


if __name__ == "__main__":
    rng = np.random.default_rng(0)
    u = rng.standard_normal((B, H, L), dtype=np.float32)
    A_re = rng.uniform(0.5, 0.99, P).astype(np.float32)
    A_im = rng.uniform(-0.5, 0.5, P).astype(np.float32)
    BC_re = rng.standard_normal((H, P), dtype=np.float32)
    BC_im = rng.standard_normal((H, P), dtype=np.float32)
    D = rng.uniform(0, 1, H).astype(np.float32)
    y = kernel(u=u, A_re=A_re, A_im=A_im, BC_re=BC_re, BC_im=BC_im, D=D)
    print("out", y.shape, y.dtype)


# revision 7
# speedup vs baseline: 33.1765x; 8.0224x over previous
"""Trainium2 Bass kernel for nn_FFTConv: y = tanh(Re(ifft(fft(u)*Ks)) + D*u).

v2 redesign:
  * Complex packing: conv with a REAL kernel commutes with Re/Im, so pack
    z[j] = u[j] + i*u[j+8] (j in [0,8)) per h -> halves all work. The real
    kernel's spectrum comes from 128 poles {A, conj(A)} with coefficients
    {BC/2, conj(BC)/2}; D*u and the 1/L ifft scale fold into the spectrum
    (delta kernel: +D to every frequency bin).
  * All matmuls bf16 (1 cyc/row), all elementwise in bf16 SBUF (DVE 2x mode),
    full 128-partition layouts via block-diagonal stationaries for the
    64-point DFT stages.
  * H-sharded across 8 cores (32 ch/core); per h: 8 packed complex rows,
    free dim 512 everywhere; 2 software-pipelined lanes.

Layout per h (L = 8192 = 64*128, n = n1 + 128*n2, m = k2 + 64*k1):
  u: [128p=(g,n2), c=2(re/im), bb=4, n1=128]   (g*4+bb = packed row)
  stage1  (PE): BD(F64) over n2       -> S1  [p=(g,k2), bb, n1]
  twiddle (DVE): * W_L^{k2 n1}
  transp  (PE): per (c,bb) 128x128    -> TP  [p=n1, c, bb, (g,k2)]
  stage2  (PE): F128 over n1          -> S2  [p=k1, bb, g, k2]
  spectral(DVE): * Ks[m]/L (+D)
  inv1    (PE): conj(F128) over k1    -> Z1  [p=o2, bb, g, k2]
  invtw   (DVE/Pool): * W_L^{+o2 k2}
  transp  (PE)                        -> TQ  [p=(g,k2), c, bb, o2]
  final   (PE): BD(conj(F64)) over k2 -> F   [p=(g,n2), bb, o2=n1]
  tanh    (Act, from PSUM)            -> y rows 0:8 = Re, 8:16 = Im
"""
import os
import sys
import numpy as np

for p in ("/opt/trn_rl_repo", "/root/.axon_site/_ro/trn_rl_repo"):
    if os.path.isdir(p) and p not in sys.path:
        sys.path.append(p)

import ml_dtypes

BF16 = ml_dtypes.bfloat16

B, H, L, P = 16, 256, 8192, 64
NCORES = 8
HSH = H // NCORES          # 32 channels per core
NLANES = int(os.environ.get("KERNEL_NLANES", "4"))
REPEAT = int(os.environ.get("KERNEL_REPEAT", "1"))
MIDBUFS = int(os.environ.get("KERNEL_MIDBUFS", "2"))
IOBUFS = int(os.environ.get("KERNEL_IOBUFS", "2"))
PFBUFS = int(os.environ.get("KERNEL_PFBUFS", "2"))
PTBUFS = int(os.environ.get("KERNEL_PTBUFS", "0"))
SKEW = int(os.environ.get("KERNEL_SKEW", "1"))
USEPOOL = os.environ.get("KERNEL_USEPOOL", "0") == "1"

_CACHE = {}


def _tables():
    a64 = np.arange(64)
    a128 = np.arange(128)
    c64 = np.cos(2 * np.pi * np.outer(a64, a64) / 64)
    s64 = np.sin(2 * np.pi * np.outer(a64, a64) / 64)
    c128 = np.cos(2 * np.pi * np.outer(a128, a128) / 128)
    s128 = np.sin(2 * np.pi * np.outer(a128, a128) / 128)
    z64 = np.zeros((64, 64))

    def bd(x):
        return np.block([[x, z64], [z64, x]])

    k2v = a128 % 64
    thT = 2 * np.pi * np.outer(k2v, a128) / L        # [(g,k2), n1]
    thI = 2 * np.pi * np.outer(a128, a64) / L        # [o2, k2]
    t2r, t2i = np.cos(thT), -np.sin(thT)
    tir, tii = np.cos(thI), np.sin(thI)
    t = {
        "bd64r": bd(c64), "bd64i": bd(-s64), "bd64in": bd(s64),
        "f128r": c128, "f128i": -s128, "f128in": s128,
        "i128": np.eye(128),
        # paired twiddle tables: [re|im] and [im|re] side by side
        "t2a": np.concatenate([t2r, t2i], axis=1),   # [128, 256]
        "t2b": np.concatenate([t2i, t2r], axis=1),
        "tia": np.concatenate([tir, tii], axis=1),   # [128, 128]
        "tib": np.concatenate([tii, tir], axis=1),
    }
    t = {k: v.astype(BF16) for k, v in t.items()}
    m = np.arange(L)
    t["cm"] = np.cos(2 * np.pi * m / L).reshape(1, L).astype(np.float32)
    t["sm"] = np.sin(2 * np.pi * m / L).reshape(1, L).astype(np.float32)
    return t


TBL_SHAPES = {
    "bd64r": [128, 128], "bd64i": [128, 128], "bd64in": [128, 128],
    "f128r": [128, 128], "f128i": [128, 128], "f128in": [128, 128],
    "i128": [128, 128], "t2a": [128, 256], "t2b": [128, 256],
    "tia": [128, 128], "tib": [128, 128],
    "cm": [1, L], "sm": [1, L],
}


def _build(nc_mod):
    bass, tile, mybir, bacc = nc_mod
    dt = mybir.dt
    f32 = dt.float32
    bf16 = dt.bfloat16
    R = dt.float32r

    def fr(ap):
        return ap.bitcast(R)

    nc = bacc.Bacc("TRN2", target_bir_lowering=False, debug=False)
    AF = mybir.ActivationFunctionType
    OP = mybir.AluOpType
    SC = 1.0 / L

    # ---------------- DRAM parameters ----------------
    # u2/y2 are host-relayouted: [p=(g,n2), h, (bb c), n1] so each h is one
    # full-width contiguous DMA (2KB/partition in, 4KB/partition out).
    u_d = nc.declare_dram_parameter("u2_sh", [128, HSH, 8, 128], bf16, isOutput=False)
    y_d = nc.declare_dram_parameter("y2_sh", [128, HSH, 8, 128], f32, isOutput=True)
    ar_d = nc.declare_dram_parameter("a_re2", [128, 1], f32, isOutput=False)
    ai_d = nc.declare_dram_parameter("a_im2", [128, 1], f32, isOutput=False)
    a2_d = nc.declare_dram_parameter("a2one", [128, 1], f32, isOutput=False)
    bcr_d = nc.declare_dram_parameter("bct_r", [128, HSH], f32, isOutput=False)
    bci_d = nc.declare_dram_parameter("bct_i", [128, HSH], f32, isOutput=False)
    bcin_d = nc.declare_dram_parameter("bct_in", [128, HSH], f32, isOutput=False)
    dl_d = nc.declare_dram_parameter("d_l", [HSH, 1], f32, isOutput=False)
    tbl_d = {}
    for n, shp in TBL_SHAPES.items():
        dty = f32 if n in ("cm", "sm") else bf16
        tbl_d[n] = nc.declare_dram_parameter(n, shp, dty, isOutput=False)

    ks_d = nc.dram_tensor("ks_scratch", [HSH, 2, L], bf16)

    with tile.TileContext(nc) as tc:
        with tc.tile_pool(name="const", bufs=1) as cpool:
            tb = {}
            for n in TBL_SHAPES:
                if n in ("cm", "sm"):
                    continue
                tb[n] = cpool.tile(TBL_SHAPES[n], bf16, tag=f"c_{n}", name=f"c_{n}")
                nc.sync.dma_start(tb[n][:], tbl_d[n][:])
            a_re = cpool.tile([128, 1], f32, tag="a_re")
            a_im = cpool.tile([128, 1], f32, tag="a_im")
            a2one = cpool.tile([128, 1], f32, tag="a2one")
            d_l = cpool.tile([HSH, 1], f32, tag="d_l")
            nc.sync.dma_start(a_re[:], ar_d[:])
            nc.sync.dma_start(a_im[:], ai_d[:])
            nc.sync.dma_start(a2one[:], a2_d[:])
            nc.sync.dma_start(d_l[:], dl_d[:])
            bct_r = cpool.tile([128, HSH], f32, tag="bct_r")
            bct_i = cpool.tile([128, HSH], f32, tag="bct_i")
            bct_in = cpool.tile([128, HSH], f32, tag="bct_in")
            nc.sync.dma_start(fr(bct_r[:]), fr(bcr_d[:]))
            nc.sync.dma_start(fr(bct_i[:]), fr(bci_d[:]))
            nc.sync.dma_start(fr(bct_in[:]), fr(bcin_d[:]))

            # ---------- prologue: G = 1/(1 - A'*W^m) for 128 poles ----------
            prologue_pools = (
                tc.tile_pool(name="gwork", bufs=1),
                tc.tile_pool(name="ksb", bufs=3),
                tc.tile_pool(name="psk", bufs=2, space=bass.MemorySpace.PSUM),
            )
            gpool = prologue_pools[0].__enter__()
            ksbp = prologue_pools[1].__enter__()
            pskp = prologue_pools[2].__enter__()

            for c in range(2):
                msl = slice(c * 4096, (c + 1) * 4096)
                ctab = gpool.tile([128, 4096], f32, tag="ctab")
                stab = gpool.tile([128, 4096], f32, tag="stab")
                nc.sync.dma_start(ctab[:], tbl_d["cm"][:, msl].broadcast_to([128, 4096]))
                nc.sync.dma_start(stab[:], tbl_d["sm"][:, msl].broadcast_to([128, 4096]))
                t1 = gpool.tile([128, 4096], f32, tag="t1")
                t2 = gpool.tile([128, 4096], f32, tag="t2")
                nc.scalar.activation(t1[:], ctab[:], AF.Identity, scale=a_re[:])
                nc.scalar.activation(t2[:], stab[:], AF.Identity, scale=a_im[:])
                eng_g = nc.gpsimd if USEPOOL else nc.vector
                q = gpool.tile([128, 4096], f32, tag="q")
                eng_g.tensor_tensor(q[:], t1[:], t2[:], OP.add)
                rn = gpool.tile([128, 4096], f32, tag="rn")
                nc.scalar.activation(rn[:], q[:], AF.Identity, scale=-2.0, bias=a2one[:])
                nc.vector.reciprocal(rn[:], rn[:])
                dr = gpool.tile([128, 4096], f32, tag="dr")
                nc.scalar.activation(dr[:], q[:], AF.Identity, scale=-1.0, bias=1.0)
                t1b = gpool.tile([128, 4096], f32, tag="t1")
                t2b = gpool.tile([128, 4096], f32, tag="t2")
                nc.scalar.activation(t1b[:], ctab[:], AF.Identity, scale=a_im[:])
                nc.scalar.activation(t2b[:], stab[:], AF.Identity, scale=a_re[:])
                di = gpool.tile([128, 4096], f32, tag="di")
                eng_g.tensor_tensor(di[:], t1b[:], t2b[:], OP.subtract)
                gr = gpool.tile([128, 4096], f32, tag="gr")
                gi = gpool.tile([128, 4096], f32, tag="gi")
                eng_g.tensor_tensor(fr(gr[:]), dr[:], rn[:], OP.mult)
                eng_g.tensor_tensor(fr(gi[:]), di[:], rn[:], OP.mult)

                for j in range(8):
                    fsl = slice(j * 512, (j + 1) * 512)
                    m0 = c * 4096 + j * 512
                    kr = pskp.tile([HSH, 512], f32, tag="kr")
                    ki = pskp.tile([HSH, 512], f32, tag="ki")
                    nc.tensor.matmul(kr[:], fr(bct_r[:]), fr(gr[:, fsl]), start=True, stop=False)
                    nc.tensor.matmul(kr[:], fr(bct_in[:]), fr(gi[:, fsl]), start=False, stop=True)
                    nc.tensor.matmul(ki[:], fr(bct_i[:]), fr(gr[:, fsl]), start=True, stop=False)
                    nc.tensor.matmul(ki[:], fr(bct_r[:]), fr(gi[:, fsl]), start=False, stop=True)
                    krs = ksbp.tile([HSH, 512], bf16, tag="krs")
                    kis = ksbp.tile([HSH, 512], bf16, tag="kis")
                    # kr/L + D/L  (folds D*u skip and ifft 1/L into the spectrum)
                    nc.scalar.activation(krs[:], kr[:], AF.Identity, scale=SC, bias=d_l[:])
                    nc.vector.tensor_scalar_mul(kis[:], ki[:], SC)
                    nc.sync.dma_start(ks_d[:, 0, m0:m0 + 512], krs[:])
                    nc.sync.dma_start(ks_d[:, 1, m0:m0 + 512], kis[:])

            prologue_pools[2].__exit__(None, None, None)
            prologue_pools[1].__exit__(None, None, None)
            prologue_pools[0].__exit__(None, None, None)

            # preload the whole spectrum [k1, h, (r,i,i,r), k2] via 4 bulk DMAs
            ks_all = cpool.tile([128, HSH, 4, 64], bf16, tag="ks_all")
            for cc, src in enumerate((0, 1, 1, 0)):
                nc.sync.dma_start(
                    ks_all[:, :, cc, :],
                    ks_d[:, src, :].rearrange("h (k1 k2) -> k1 h k2", k2=64))

            # ---------- main loop ----------
            main_pools = [
                tc.tile_pool(name="io", bufs=IOBUFS),
                tc.tile_pool(name="mid", bufs=MIDBUFS),
                tc.tile_pool(name="pf", bufs=PFBUFS, space=bass.MemorySpace.PSUM),
            ]
            if PTBUFS > 0:
                main_pools.append(
                    tc.tile_pool(name="pt", bufs=PTBUFS, space=bass.MemorySpace.PSUM))
            iop = main_pools[0].__enter__()
            midp = main_pools[1].__enter__()
            pfp = main_pools[2].__enter__()
            ptp = main_pools[3].__enter__() if PTBUFS > 0 else None

            t2a_b = tb["t2a"][:].unsqueeze(1).broadcast_to([128, 4, 256])
            t2b_b = tb["t2b"][:].unsqueeze(1).broadcast_to([128, 4, 256])
            tia_b = tb["tia"][:].unsqueeze(1).broadcast_to([128, 8, 128])
            tib_b = tb["tib"][:].unsqueeze(1).broadcast_to([128, 8, 128])

            def stages(h, lane):
                sfx = str(lane)

                def mtile(tag, w=512):
                    return midp.tile([128, w], bf16, tag=tag + sfx, name=tag)

                # --- loads ---
                ks_a = ks_all[:, h, 0:2, :].rearrange("p a b -> p (a b)").unsqueeze(
                    1).broadcast_to([128, 8, 128])
                ks_b = ks_all[:, h, 2:4, :].rearrange("p a b -> p (a b)").unsqueeze(
                    1).broadcast_to([128, 8, 128])
                # packing: complex row j = g*4+bb is u[2j] + i*u[2j+1];
                # SBUF layout [p, bb, c, n1]; host pre-layouts u2 so one
                # full-width contiguous DMA loads the whole h.
                uc = iop.tile([128, 2, 512], bf16, tag="uc" + sfx, name="uc")
                nc.sync.dma_start(
                    uc[:].rearrange("p c (bb n1) -> p (c bb) n1", n1=128), u_d[:, h])
                yield

                # --- stage 1: BD(F64) over n2 ---
                S1i = pfp.tile([128, 512], f32, tag="pf" + sfx, name="S1i")
                S1r = pfp.tile([128, 512], f32, tag="pf" + sfx, name="S1r")
                ure, uim = uc[:, 0], uc[:, 1]
                nc.tensor.matmul(S1i[:], tb["bd64i"][:], ure, start=True, stop=False)
                nc.tensor.matmul(S1i[:], tb["bd64r"][:], uim, start=False, stop=True)
                nc.tensor.matmul(S1r[:], tb["bd64r"][:], ure, start=True, stop=False)
                nc.tensor.matmul(S1r[:], tb["bd64in"][:], uim, start=False, stop=True)
                # y1 layout [p, bb, (c n1)]
                y1 = midp.tile([128, 4, 256], bf16, tag="y1" + sfx, name="y1")
                nc.scalar.activation(
                    y1[:, :, 0:128], S1r[:].rearrange("p (a b) -> p a b", a=4), AF.Copy)
                nc.scalar.activation(
                    y1[:, :, 128:256], S1i[:].rearrange("p (a b) -> p a b", a=4), AF.Copy)
                yield

                # --- fwd twiddle: 2 paired mults (DVE) + 2 addsubs ---
                ma = midp.tile([128, 4, 256], bf16, tag="mA" + sfx, name="ma")
                mb = midp.tile([128, 4, 256], bf16, tag="mB" + sfx, name="mb")
                nc.vector.tensor_tensor(ma[:], y1[:], t2a_b, OP.mult)
                nc.vector.tensor_tensor(mb[:], y1[:], t2b_b, OP.mult)
                y2r = ma[:, :, 0:128]
                y2i = mb[:, :, 0:128]
                eng_as = nc.gpsimd if USEPOOL else nc.vector
                eng_as.tensor_tensor(y2r, ma[:, :, 0:128], ma[:, :, 128:256], OP.subtract)
                eng_as.tensor_tensor(y2i, mb[:, :, 0:128], mb[:, :, 128:256], OP.add)
                yield

                # --- fwd transposes ---
                TP = ptp.tile([128, 2, 512], bf16, tag="pt" + sfx, name="TP") \
                    if ptp is not None else \
                    pfp.tile([128, 2, 512], bf16, tag="pf" + sfx, name="TP")
                for cc, srcv in ((0, y2r), (1, y2i)):
                    tpv = TP[:, cc].rearrange("p (a b) -> p a b", a=4)
                    for bb in range(4):
                        nc.tensor.transpose(tpv[:, bb, :], srcv[:, bb, :], tb["i128"][:])
                y2t = midp.tile([128, 2, 512], bf16, tag="y2t" + sfx, name="y2t")
                nc.vector.tensor_scalar_add(
                    y2t[:].rearrange("p a b -> p (a b)"),
                    TP[:].rearrange("p a b -> p (a b)"), 0.0)
                yield

                # --- stage 2: F128 over n1 ---
                S2i = pfp.tile([128, 512], f32, tag="pf" + sfx, name="S2i")
                S2r = pfp.tile([128, 512], f32, tag="pf" + sfx, name="S2r")
                y2tr, y2ti = y2t[:, 0], y2t[:, 1]
                nc.tensor.matmul(S2i[:], tb["f128i"][:], y2tr, start=True, stop=False)
                nc.tensor.matmul(S2i[:], tb["f128r"][:], y2ti, start=False, stop=True)
                nc.tensor.matmul(S2r[:], tb["f128r"][:], y2tr, start=True, stop=False)
                nc.tensor.matmul(S2r[:], tb["f128in"][:], y2ti, start=False, stop=True)
                # x layout [p, (bb g), (c k2)]
                x = midp.tile([128, 8, 128], bf16, tag="x" + sfx, name="x")
                nc.scalar.activation(
                    x[:, :, 0:64], S2r[:].rearrange("p (a b) -> p a b", a=8), AF.Copy)
                nc.scalar.activation(
                    x[:, :, 64:128], S2i[:].rearrange("p (a b) -> p a b", a=8), AF.Copy)
                yield

                # --- spectral multiply: 2 paired mults (DVE) + 2 addsubs (Pool) ---
                ma = midp.tile([128, 8, 128], bf16, tag="mA" + sfx, name="ma")
                mb = midp.tile([128, 8, 128], bf16, tag="mB" + sfx, name="mb")
                nc.vector.tensor_tensor(ma[:], x[:], ks_a, OP.mult)
                nc.vector.tensor_tensor(mb[:], x[:], ks_b, OP.mult)
                sre = midp.tile([128, 512], bf16, tag="w1" + sfx, name="sre")
                sim_ = midp.tile([128, 512], bf16, tag="w2" + sfx, name="sim")
                s_re, s_im = sre[:], sim_[:]
                eng_as.tensor_tensor(
                    sre[:].rearrange("p (a b) -> p a b", a=8),
                    ma[:, :, 0:64], ma[:, :, 64:128], OP.subtract)
                eng_as.tensor_tensor(
                    sim_[:].rearrange("p (a b) -> p a b", a=8),
                    mb[:, :, 0:64], mb[:, :, 64:128], OP.add)
                yield

                # --- inverse stage 1: conj(F128) over k1 ---
                Z1i = pfp.tile([128, 512], f32, tag="pf" + sfx, name="Z1i")
                Z1r = pfp.tile([128, 512], f32, tag="pf" + sfx, name="Z1r")
                nc.tensor.matmul(Z1i[:], tb["f128in"][:], s_re, start=True, stop=False)
                nc.tensor.matmul(Z1i[:], tb["f128r"][:], s_im, start=False, stop=True)
                nc.tensor.matmul(Z1r[:], tb["f128r"][:], s_re, start=True, stop=False)
                nc.tensor.matmul(Z1r[:], tb["f128i"][:], s_im, start=False, stop=True)
                # z1 layout [p, (bb g), (c k2)]
                z1 = midp.tile([128, 8, 128], bf16, tag="z1" + sfx, name="z1")
                nc.scalar.activation(
                    z1[:, :, 0:64], Z1r[:].rearrange("p (a b) -> p a b", a=8), AF.Copy)
                nc.scalar.activation(
                    z1[:, :, 64:128], Z1i[:].rearrange("p (a b) -> p a b", a=8), AF.Copy)
                yield

                # --- inverse twiddle: 2 paired mults (DVE) + 2 addsubs (Pool) ---
                ma = midp.tile([128, 8, 128], bf16, tag="mA" + sfx, name="ma")
                mb = midp.tile([128, 8, 128], bf16, tag="mB" + sfx, name="mb")
                nc.vector.tensor_tensor(ma[:], z1[:], tia_b, OP.mult)
                nc.vector.tensor_tensor(mb[:], z1[:], tib_b, OP.mult)
                z2r = midp.tile([128, 512], bf16, tag="w1" + sfx, name="z2r")
                z2i = midp.tile([128, 512], bf16, tag="w2" + sfx, name="z2i")
                eng_as.tensor_tensor(
                    z2r[:].rearrange("p (a b) -> p a b", a=8),
                    ma[:, :, 0:64], ma[:, :, 64:128], OP.subtract)
                eng_as.tensor_tensor(
                    z2i[:].rearrange("p (a b) -> p a b", a=8),
                    mb[:, :, 0:64], mb[:, :, 64:128], OP.add)
                yield

                # --- inverse transposes ---
                TQ = ptp.tile([128, 2, 512], bf16, tag="pt" + sfx, name="TQ") \
                    if ptp is not None else \
                    pfp.tile([128, 2, 512], bf16, tag="pf" + sfx, name="TQ")
                for cc, zsrc in ((0, z2r), (1, z2i)):
                    tqv = TQ[:, cc].rearrange("p (a b) -> p a b", a=4)
                    zsv = zsrc[:].rearrange("p (a b) -> p a b", a=4)
                    for bb in range(4):
                        nc.tensor.transpose(tqv[:, bb, :], zsv[:, bb, :], tb["i128"][:])
                z2t = midp.tile([128, 2, 512], bf16, tag="z2t" + sfx, name="z2t")
                nc.vector.tensor_scalar_add(
                    z2t[:].rearrange("p a b -> p (a b)"),
                    TQ[:].rearrange("p a b -> p (a b)"), 0.0)
                yield

                # --- final: BD(conj(F64)) over k2, tanh, store ---
                Fi = pfp.tile([128, 512], f32, tag="pf" + sfx, name="Fi")
                Fr = pfp.tile([128, 512], f32, tag="pf" + sfx, name="Fr")
                z2tr, z2ti = z2t[:, 0], z2t[:, 1]
                nc.tensor.matmul(Fi[:], tb["bd64in"][:], z2tr, start=True, stop=False)
                nc.tensor.matmul(Fi[:], tb["bd64r"][:], z2ti, start=False, stop=True)
                nc.tensor.matmul(Fr[:], tb["bd64r"][:], z2tr, start=True, stop=False)
                nc.tensor.matmul(Fr[:], tb["bd64i"][:], z2ti, start=False, stop=True)
                yo = iop.tile([128, 2, 512], f32, tag="yo" + sfx, name="yo")
                nc.scalar.activation(yo[:, 0], Fr[:], AF.Tanh)
                nc.scalar.activation(yo[:, 1], Fi[:], AF.Tanh)
                nc.sync.dma_start(
                    y_d[:, h], yo[:].rearrange("p c (bb n1) -> p (c bb) n1", n1=128))
                yield

            def lane_stream(ln):
                for _rep in range(REPEAT):
                    for h in range(ln, HSH, NLANES):
                        yield from stages(h, ln)

            gens = [lane_stream(ln) for ln in range(NLANES)]
            done = [False] * NLANES
            # prime lanes with a stage skew so engine queues interleave
            # different pipeline stages instead of running in lockstep
            for ln in range(NLANES):
                for _ in range((NLANES - 1 - ln) * SKEW):
                    try:
                        next(gens[ln])
                    except StopIteration:
                        done[ln] = True
                        break
            while not all(done):
                for gi_, g in enumerate(gens):
                    if not done[gi_]:
                        try:
                            next(g)
                        except StopIteration:
                            done[gi_] = True

            for mp in reversed(main_pools):
                mp.__exit__(None, None, None)

    nc.compile()
    return nc


def _get_program():
    key = ("prog", NLANES, REPEAT, MIDBUFS, IOBUFS, PFBUFS, PTBUFS)
    if key not in _CACHE:
        import concourse.bass as bass
        import concourse.tile as tile
        from concourse import mybir, bacc
        _CACHE[key] = _build((bass, tile, mybir, bacc))
    return _CACHE[key]


def _u_relayout(u_bf_core):
    """[16, HSH, L] -> [p=(g,n2), h, (c bb), n1]  (b = g*8 + bb*2 + c)."""
    us = u_bf_core.reshape(2, 4, 2, HSH, 64, 128)   # g bb c h n2 n1
    return np.ascontiguousarray(us.transpose(0, 4, 3, 2, 1, 5).reshape(128, HSH, 8, 128))


def _y_relayout(y2_core):
    """[p=(g,n2), h, (c bb), n1] -> [16, HSH, L]."""
    ys = y2_core.reshape(2, 64, HSH, 2, 4, 128)      # g n2 h c bb n1
    return ys.transpose(0, 4, 3, 2, 1, 5).reshape(16, HSH, L)


def make_in_maps(u, A_re, A_im, BC_re, BC_im, D):
    tabs = _tables()
    u_bf = np.ascontiguousarray(u).astype(BF16)
    a_re2 = np.concatenate([A_re, A_re]).reshape(128, 1).astype(np.float32)
    a_im2 = np.concatenate([A_im, -A_im]).reshape(128, 1).astype(np.float32)
    a2one = (1.0 + A_re.astype(np.float64) ** 2 + A_im.astype(np.float64) ** 2)
    a2one = np.concatenate([a2one, a2one]).reshape(128, 1).astype(np.float32)
    in_maps = []
    for c in range(NCORES):
        hs = slice(c * HSH, (c + 1) * HSH)
        bcr = BC_re[hs].T.astype(np.float32) / 2
        bci = BC_im[hs].T.astype(np.float32) / 2
        m = {
            "u2_sh": _u_relayout(u_bf[:, hs, :]),
            "a_re2": a_re2,
            "a_im2": a_im2,
            "a2one": a2one,
            "bct_r": np.ascontiguousarray(np.concatenate([bcr, bcr], axis=0)),
            "bct_i": np.ascontiguousarray(np.concatenate([bci, -bci], axis=0)),
            "bct_in": np.ascontiguousarray(np.concatenate([-bci, bci], axis=0)),
            "d_l": np.ascontiguousarray(
                (D[hs] / L).reshape(HSH, 1).astype(np.float32)),
        }
        m.update(tabs)
        in_maps.append(m)
    return in_maps


def kernel(u, A_re, A_im, BC_re, BC_im, D):
    from concourse.bass_utils import run_bass_kernel_spmd

    nc = _get_program()
    in_maps = make_in_maps(u, A_re, A_im, BC_re, BC_im, D)

    res = None
    last_err = None
    for attempt in range(3):
        try:
            res = run_bass_kernel_spmd(nc, in_maps, list(range(NCORES)))
            break
        except Exception as e:  # transient NRT_EXEC_UNIT_UNRECOVERABLE flakes
            last_err = e
            import time as _time
            _time.sleep(2.0)
    if res is None:
        raise last_err
    out = np.concatenate(
        [_y_relayout(res.results[c]["y2_sh"]) for c in range(NCORES)], axis=1)
    return np.ascontiguousarray(out, dtype=np.float32)


if __name__ == "__main__":
    rng = np.random.default_rng(0)
    u = rng.standard_normal((B, H, L), dtype=np.float32)
    A_re = rng.uniform(0.5, 0.99, P).astype(np.float32)
    A_im = rng.uniform(-0.5, 0.5, P).astype(np.float32)
    BC_re = rng.standard_normal((H, P), dtype=np.float32)
    BC_im = rng.standard_normal((H, P), dtype=np.float32)
    D = rng.uniform(0, 1, H).astype(np.float32)
    y = kernel(u=u, A_re=A_re, A_im=A_im, BC_re=BC_re, BC_im=BC_im, D=D)
    print("out", y.shape, y.dtype)


# revision 13
# speedup vs baseline: 70.1291x; 2.1138x over previous
"""Trainium2 Bass kernel for nn_FFTConv: y = tanh(Re(ifft(fft(u)*Ks)) + D*u).

v3 design:
  * Complex packing: conv with a REAL kernel commutes with Re/Im, so pack
    z[j] = u[2j] + i*u[2j+1] (j in [0,8)) per h -> halves all work. The
    real kernel's spectrum Ks (with D*u and the 1/L ifft scale folded in)
    is computed on the HOST from the 128 poles {A, conj(A)} and shipped as
    a bf16 parameter in its final on-chip layout -- no device prologue.
  * All matmuls bf16 (1 cyc/row), elementwise in bf16 SBUF (DVE 2x mode),
    full 128-partition layouts via block-diagonal stationaries for the
    64-point DFT stages. GPSIMD is avoided entirely (HW tensor ops there
    are ~20x slower than any model).
  * Fwd-twiddle add/sub folded into the corner-turn transposes via +/-I
    accumulation on the PE.
  * H-sharded over 8 cores (32 ch/core); per h: 8 packed complex rows,
    free dim 512 everywhere; NLANES software-pipelined lanes with skew.

Per-h dataflow (L = 8192 = 64*128, n = n1 + 128*n2, m = k2 + 64*k1):
  u:      [p=(g,n2), (c bb), n1]  one contiguous DMA (host pre-layout)
  stage1  (PE)  BD(F64) over n2        -> S1r,S1i [p=(g,k2), bb, n1]
  y1 copy (Act) PSUM->SBUF bf16 as [p, bb, (c n1)]
  fwd tw  (DVE) 2 paired mults ma=y1*[t2r|t2i], mb=y1*[t2i|t2r]
  transp  (PE)  16x 128x128, re = ma0.T - ma1.T, im = mb0.T + mb1.T
  y2t     (DVE) PSUM->SBUF bf16
  stage2  (PE)  F128 over n1           -> S2r,S2i [p=k1, (bb g), k2]
  x copy  (Act) PSUM->SBUF bf16 as [p, (bb g), (c k2)]
  spectral(DVE) 2 paired mults vs Ks[(r,i)|(i,r)] + 2 addsubs
  inv1    (PE)  conj(F128) over k1     -> Z1r,Z1i [p=o2, (bb g), k2]
  z1 copy (Act)
  inv tw  (DVE) 2 paired mults + 2 addsubs
  transp  (PE)  8x
  z2t     (Act) PSUM->SBUF bf16
  final   (PE)  BD(conj(F64)) over k2  -> Fr,Fi [p=(g,n2), bb, n1]
  tanh    (Act) from PSUM -> yo f32; one contiguous DMA out
"""
import os
import sys
import numpy as np

for p in ("/opt/trn_rl_repo", "/root/.axon_site/_ro/trn_rl_repo"):
    if os.path.isdir(p) and p not in sys.path:
        sys.path.append(p)

import ml_dtypes

BF16 = ml_dtypes.bfloat16

B, H, L, P = 16, 256, 8192, 64
NCORES = 8
HSH = H // NCORES          # 32 channels per core
NLANES = int(os.environ.get("KERNEL_NLANES", "4"))
REPEAT = int(os.environ.get("KERNEL_REPEAT", "1"))
MIDBUFS = int(os.environ.get("KERNEL_MIDBUFS", "2"))
IOBUFS = int(os.environ.get("KERNEL_IOBUFS", "2"))
PFBUFS = int(os.environ.get("KERNEL_PFBUFS", "2"))
PTBUFS = int(os.environ.get("KERNEL_PTBUFS", "0"))
SKEW = int(os.environ.get("KERNEL_SKEW", "1"))

_CACHE = {}


def _tables():
    a64 = np.arange(64)
    a128 = np.arange(128)
    c64 = np.cos(2 * np.pi * np.outer(a64, a64) / 64)
    s64 = np.sin(2 * np.pi * np.outer(a64, a64) / 64)
    c128 = np.cos(2 * np.pi * np.outer(a128, a128) / 128)
    s128 = np.sin(2 * np.pi * np.outer(a128, a128) / 128)
    z64 = np.zeros((64, 64))

    def bd(x):
        return np.block([[x, z64], [z64, x]])

    k2v = a128 % 64
    thT = 2 * np.pi * np.outer(k2v, a128) / L        # [(g,k2), n1]
    thI = 2 * np.pi * np.outer(a128, a64) / L        # [o2, k2]
    t2r, t2i = np.cos(thT), -np.sin(thT)
    tir, tii = np.cos(thI), np.sin(thI)
    t = {
        "bd64r": bd(c64), "bd64i": bd(-s64), "bd64in": bd(s64),
        "f128r": c128, "f128i": -s128, "f128in": s128,
        "i128": np.eye(128), "i128n": -np.eye(128),
        # paired twiddle tables: [re|im] and [im|re] side by side
        "t2a": np.concatenate([t2r, t2i], axis=1),   # [128, 256]
        "t2b": np.concatenate([t2i, t2r], axis=1),
        "tia": np.concatenate([tir, tii], axis=1),   # [128, 128]
        "tib": np.concatenate([tii, tir], axis=1),
    }
    return {k: v.astype(BF16) for k, v in t.items()}


TBL_SHAPES = {
    "bd64r": [128, 128], "bd64i": [128, 128], "bd64in": [128, 128],
    "f128r": [128, 128], "f128i": [128, 128], "f128in": [128, 128],
    "i128": [128, 128], "i128n": [128, 128],
    "t2a": [128, 256], "t2b": [128, 256],
    "tia": [128, 128], "tib": [128, 128],
}


def _build(nc_mod):
    bass, tile, mybir, bacc = nc_mod
    dt = mybir.dt
    f32 = dt.float32
    bf16 = dt.bfloat16

    nc = bacc.Bacc("TRN2", target_bir_lowering=False, debug=False)
    AF = mybir.ActivationFunctionType
    OP = mybir.AluOpType

    # ---------------- DRAM parameters ----------------
    # u2/y2 host-relayouted: [p=(g,n2), h, (c bb), n1] -> one contiguous
    # full-width DMA per h. ks2 is the host-computed spectrum in its final
    # on-chip layout [k1, h, (r,i,i,r), k2].
    u_d = nc.declare_dram_parameter("u2_sh", [128, HSH, 8, 128], bf16, isOutput=False)
    y_d = nc.declare_dram_parameter("y2_sh", [128, HSH, 8, 128], f32, isOutput=True)
    ks2_d = nc.declare_dram_parameter("ks2_sh", [128, HSH, 4, 64], bf16, isOutput=False)
    tbl_d = {n: nc.declare_dram_parameter(n, shp, bf16, isOutput=False)
             for n, shp in TBL_SHAPES.items()}

    with tile.TileContext(nc) as tc:
        with tc.tile_pool(name="const", bufs=1) as cpool:
            tb = {}
            for n in TBL_SHAPES:
                tb[n] = cpool.tile(TBL_SHAPES[n], bf16, tag=f"c_{n}", name=f"c_{n}")
                nc.sync.dma_start(tb[n][:], tbl_d[n][:])
            ks_all = cpool.tile([128, HSH, 4, 64], bf16, tag="ks_all")
            nc.sync.dma_start(
                ks_all[:].rearrange("p a b c -> p (a b c)"),
                ks2_d[:].rearrange("p a b c -> p (a b c)"))

            main_pools = [
                tc.tile_pool(name="io", bufs=IOBUFS),
                tc.tile_pool(name="mid", bufs=MIDBUFS),
                tc.tile_pool(name="pf", bufs=PFBUFS, space=bass.MemorySpace.PSUM),
            ]
            if PTBUFS > 0:
                main_pools.append(
                    tc.tile_pool(name="pt", bufs=PTBUFS, space=bass.MemorySpace.PSUM))
            iop = main_pools[0].__enter__()
            midp = main_pools[1].__enter__()
            pfp = main_pools[2].__enter__()
            ptp = main_pools[3].__enter__() if PTBUFS > 0 else None

            t2a_b = tb["t2a"][:].unsqueeze(1).broadcast_to([128, 4, 256])
            t2b_b = tb["t2b"][:].unsqueeze(1).broadcast_to([128, 4, 256])
            tia_b = tb["tia"][:].unsqueeze(1).broadcast_to([128, 8, 128])
            tib_b = tb["tib"][:].unsqueeze(1).broadcast_to([128, 8, 128])

            def stages(h, lane):
                sfx = str(lane)

                ks_a = ks_all[:, h, 0:2, :].rearrange("p a b -> p (a b)").unsqueeze(
                    1).broadcast_to([128, 8, 128])
                ks_b = ks_all[:, h, 2:4, :].rearrange("p a b -> p (a b)").unsqueeze(
                    1).broadcast_to([128, 8, 128])

                # --- load: one contiguous full-width DMA ---
                uc = iop.tile([128, 2, 512], bf16, tag="uc" + sfx, name="uc")
                nc.sync.dma_start(
                    uc[:].rearrange("p c (bb n1) -> p (c bb) n1", n1=128), u_d[:, h])
                yield

                # --- stage 1: BD(F64) over n2 ---
                S1i = pfp.tile([128, 512], f32, tag="pf" + sfx, name="S1i")
                S1r = pfp.tile([128, 512], f32, tag="pf" + sfx, name="S1r")
                ure, uim = uc[:, 0], uc[:, 1]
                nc.tensor.matmul(S1i[:], tb["bd64i"][:], ure, start=True, stop=False)
                nc.tensor.matmul(S1i[:], tb["bd64r"][:], uim, start=False, stop=True)
                nc.tensor.matmul(S1r[:], tb["bd64r"][:], ure, start=True, stop=False)
                nc.tensor.matmul(S1r[:], tb["bd64in"][:], uim, start=False, stop=True)
                # y1 layout [p, bb, (c n1)]
                y1 = midp.tile([128, 4, 256], bf16, tag="y1" + sfx, name="y1")
                nc.scalar.activation(
                    y1[:, :, 0:128], S1r[:].rearrange("p (a b) -> p a b", a=4), AF.Copy)
                nc.scalar.activation(
                    y1[:, :, 128:256], S1i[:].rearrange("p (a b) -> p a b", a=4), AF.Copy)
                yield

                # --- fwd twiddle: 2 paired mults + 2 addsubs (DVE) ---
                ma = midp.tile([128, 4, 256], bf16, tag="mA" + sfx, name="ma")
                mb = midp.tile([128, 4, 256], bf16, tag="mB" + sfx, name="mb")
                nc.vector.tensor_tensor(ma[:], y1[:], t2a_b, OP.mult)
                nc.vector.tensor_tensor(mb[:], y1[:], t2b_b, OP.mult)
                y2r = midp.tile([128, 512], bf16, tag="w1" + sfx, name="y2r")
                y2i = midp.tile([128, 512], bf16, tag="w2" + sfx, name="y2i")
                nc.vector.tensor_tensor(
                    y2r[:].rearrange("p (a b) -> p a b", a=4),
                    ma[:, :, 0:128], ma[:, :, 128:256], OP.subtract)
                nc.vector.tensor_tensor(
                    y2i[:].rearrange("p (a b) -> p a b", a=4),
                    mb[:, :, 0:128], mb[:, :, 128:256], OP.add)
                yield

                # --- fwd transposes ---
                TP = ptp.tile([128, 2, 512], bf16, tag="pt" + sfx, name="TP") \
                    if ptp is not None else \
                    pfp.tile([128, 2, 512], bf16, tag="pf" + sfx, name="TP")
                for cc, src in ((0, y2r), (1, y2i)):
                    tpv = TP[:, cc].rearrange("p (a b) -> p a b", a=4)
                    srcv = src[:].rearrange("p (a b) -> p a b", a=4)
                    for bb in range(4):
                        nc.tensor.transpose(tpv[:, bb, :], srcv[:, bb, :], tb["i128"][:])
                y2t = midp.tile([128, 2, 512], bf16, tag="y2t" + sfx, name="y2t")
                nc.vector.tensor_scalar_add(
                    y2t[:].rearrange("p a b -> p (a b)"),
                    TP[:].rearrange("p a b -> p (a b)"), 0.0)
                yield

                # --- stage 2: F128 over n1 ---
                S2i = pfp.tile([128, 512], f32, tag="pf" + sfx, name="S2i")
                S2r = pfp.tile([128, 512], f32, tag="pf" + sfx, name="S2r")
                y2tr, y2ti = y2t[:, 0], y2t[:, 1]
                nc.tensor.matmul(S2i[:], tb["f128i"][:], y2tr, start=True, stop=False)
                nc.tensor.matmul(S2i[:], tb["f128r"][:], y2ti, start=False, stop=True)
                nc.tensor.matmul(S2r[:], tb["f128r"][:], y2tr, start=True, stop=False)
                nc.tensor.matmul(S2r[:], tb["f128in"][:], y2ti, start=False, stop=True)
                # x layout [p, (bb g), (c k2)]
                x = midp.tile([128, 8, 128], bf16, tag="x" + sfx, name="x")
                nc.scalar.activation(
                    x[:, :, 0:64], S2r[:].rearrange("p (a b) -> p a b", a=8), AF.Copy)
                nc.scalar.activation(
                    x[:, :, 64:128], S2i[:].rearrange("p (a b) -> p a b", a=8), AF.Copy)
                yield

                # --- spectral multiply: 2 paired mults + 2 addsubs (DVE) ---
                ma = midp.tile([128, 8, 128], bf16, tag="mA" + sfx, name="ma")
                mb = midp.tile([128, 8, 128], bf16, tag="mB" + sfx, name="mb")
                nc.vector.tensor_tensor(ma[:], x[:], ks_a, OP.mult)
                nc.vector.tensor_tensor(mb[:], x[:], ks_b, OP.mult)
                sre = midp.tile([128, 512], bf16, tag="w1" + sfx, name="sre")
                sim_ = midp.tile([128, 512], bf16, tag="w2" + sfx, name="sim")
                s_re, s_im = sre[:], sim_[:]
                nc.vector.tensor_tensor(
                    sre[:].rearrange("p (a b) -> p a b", a=8),
                    ma[:, :, 0:64], ma[:, :, 64:128], OP.subtract)
                nc.vector.tensor_tensor(
                    sim_[:].rearrange("p (a b) -> p a b", a=8),
                    mb[:, :, 0:64], mb[:, :, 64:128], OP.add)
                yield

                # --- inverse stage 1: conj(F128) over k1 ---
                Z1i = pfp.tile([128, 512], f32, tag="pf" + sfx, name="Z1i")
                Z1r = pfp.tile([128, 512], f32, tag="pf" + sfx, name="Z1r")
                nc.tensor.matmul(Z1i[:], tb["f128in"][:], s_re, start=True, stop=False)
                nc.tensor.matmul(Z1i[:], tb["f128r"][:], s_im, start=False, stop=True)
                nc.tensor.matmul(Z1r[:], tb["f128r"][:], s_re, start=True, stop=False)
                nc.tensor.matmul(Z1r[:], tb["f128i"][:], s_im, start=False, stop=True)
                # z1 layout [p, (bb g), (c k2)]
                z1 = midp.tile([128, 8, 128], bf16, tag="z1" + sfx, name="z1")
                nc.scalar.activation(
                    z1[:, :, 0:64], Z1r[:].rearrange("p (a b) -> p a b", a=8), AF.Copy)
                nc.scalar.activation(
                    z1[:, :, 64:128], Z1i[:].rearrange("p (a b) -> p a b", a=8), AF.Copy)
                yield

                # --- inverse twiddle: 2 paired mults + 2 addsubs (DVE) ---
                ma = midp.tile([128, 8, 128], bf16, tag="mA" + sfx, name="ma")
                mb = midp.tile([128, 8, 128], bf16, tag="mB" + sfx, name="mb")
                nc.vector.tensor_tensor(ma[:], z1[:], tia_b, OP.mult)
                nc.vector.tensor_tensor(mb[:], z1[:], tib_b, OP.mult)
                z2r = midp.tile([128, 512], bf16, tag="w1" + sfx, name="z2r")
                z2i = midp.tile([128, 512], bf16, tag="w2" + sfx, name="z2i")
                nc.vector.tensor_tensor(
                    z2r[:].rearrange("p (a b) -> p a b", a=8),
                    ma[:, :, 0:64], ma[:, :, 64:128], OP.subtract)
                nc.vector.tensor_tensor(
                    z2i[:].rearrange("p (a b) -> p a b", a=8),
                    mb[:, :, 0:64], mb[:, :, 64:128], OP.add)
                yield

                # --- inverse transposes ---
                TQ = ptp.tile([128, 2, 512], bf16, tag="pt" + sfx, name="TQ") \
                    if ptp is not None else \
                    pfp.tile([128, 2, 512], bf16, tag="pf" + sfx, name="TQ")
                for cc, zsrc in ((0, z2r), (1, z2i)):
                    tqv = TQ[:, cc].rearrange("p (a b) -> p a b", a=4)
                    zsv = zsrc[:].rearrange("p (a b) -> p a b", a=4)
                    for bb in range(4):
                        nc.tensor.transpose(tqv[:, bb, :], zsv[:, bb, :], tb["i128"][:])
                z2t = midp.tile([128, 2, 512], bf16, tag="z2t" + sfx, name="z2t")
                nc.scalar.activation(
                    z2t[:].rearrange("p a b -> p (a b)"),
                    TQ[:].rearrange("p a b -> p (a b)"), AF.Copy)
                yield

                # --- final: BD(conj(F64)) over k2, tanh, store ---
                Fi = pfp.tile([128, 512], f32, tag="pf" + sfx, name="Fi")
                Fr = pfp.tile([128, 512], f32, tag="pf" + sfx, name="Fr")
                z2tr, z2ti = z2t[:, 0], z2t[:, 1]
                nc.tensor.matmul(Fi[:], tb["bd64in"][:], z2tr, start=True, stop=False)
                nc.tensor.matmul(Fi[:], tb["bd64r"][:], z2ti, start=False, stop=True)
                nc.tensor.matmul(Fr[:], tb["bd64r"][:], z2tr, start=True, stop=False)
                nc.tensor.matmul(Fr[:], tb["bd64i"][:], z2ti, start=False, stop=True)
                yo = iop.tile([128, 2, 512], f32, tag="yo" + sfx, name="yo")
                nc.scalar.activation(yo[:, 0], Fr[:], AF.Tanh)
                nc.scalar.activation(yo[:, 1], Fi[:], AF.Tanh)
                nc.sync.dma_start(
                    y_d[:, h], yo[:].rearrange("p c (bb n1) -> p (c bb) n1", n1=128))
                yield

            def lane_stream(ln):
                for _rep in range(REPEAT):
                    for h in range(ln, HSH, NLANES):
                        yield from stages(h, ln)

            gens = [lane_stream(ln) for ln in range(NLANES)]
            done = [False] * NLANES
            # prime lanes with a stage skew so engine queues interleave
            # different pipeline stages instead of running in lockstep
            for ln in range(NLANES):
                for _ in range((NLANES - 1 - ln) * SKEW):
                    try:
                        next(gens[ln])
                    except StopIteration:
                        done[ln] = True
                        break
            while not all(done):
                for gi_, g in enumerate(gens):
                    if not done[gi_]:
                        try:
                            next(g)
                        except StopIteration:
                            done[gi_] = True

            for mp in reversed(main_pools):
                mp.__exit__(None, None, None)

    nc.compile()
    return nc


def _get_program():
    key = ("prog", NLANES, REPEAT, MIDBUFS, IOBUFS, PFBUFS, PTBUFS, SKEW)
    if key not in _CACHE:
        import concourse.bass as bass
        import concourse.tile as tile
        from concourse import mybir, bacc
        _CACHE[key] = _build((bass, tile, mybir, bacc))
    return _CACHE[key]


def _u_relayout(u_bf_core):
    """[16, HSH, L] -> [p=(g,n2), h, (c bb), n1]  (b = g*8 + bb*2 + c)."""
    us = u_bf_core.reshape(2, 4, 2, HSH, 64, 128)   # g bb c h n2 n1
    return np.ascontiguousarray(us.transpose(0, 4, 3, 2, 1, 5).reshape(128, HSH, 8, 128))


def _y_relayout(y2_core):
    """[p=(g,n2), h, (c bb), n1] -> [16, HSH, L]."""
    ys = y2_core.reshape(2, 64, HSH, 2, 4, 128)      # g n2 h c bb n1
    return ys.transpose(0, 4, 3, 2, 1, 5).reshape(16, HSH, L)


def make_in_maps(u, A_re, A_im, BC_re, BC_im, D):
    tabs = _tables()
    u_bf = np.ascontiguousarray(u).astype(BF16)
    # host-side kernel spectrum: Ks[h,m] = (sum_p c'_p/(1-A'_p W^m) + D[h])/L
    m = np.arange(L)
    W = np.exp(-2j * np.pi * m / L).astype(np.complex64)
    A2 = np.concatenate([A_re + 1j * A_im, A_re - 1j * A_im]).astype(np.complex64)
    G = (1.0 / (1.0 - A2[:, None] * W[None, :])).astype(np.complex64)   # (128, L)
    BC = (BC_re + 1j * BC_im).astype(np.complex64)
    in_maps = []
    for c in range(NCORES):
        hs = slice(c * HSH, (c + 1) * HSH)
        C2 = np.concatenate([BC[hs] / 2, np.conj(BC[hs]) / 2], axis=1)  # (HSH, 128)
        Ks = (C2 @ G + D[hs].astype(np.complex64)[:, None]) / np.float32(L)  # (HSH, L)
        kr = Ks.real.astype(BF16).reshape(HSH, 128, 64)   # [h, k1, k2]
        ki = Ks.imag.astype(BF16).reshape(HSH, 128, 64)
        ks2 = np.stack([kr, ki, ki, kr], axis=2)          # [h, k1, c, k2]
        ks2 = np.ascontiguousarray(ks2.transpose(1, 0, 2, 3))  # [k1, h, c, k2]
        m_ = {
            "u2_sh": _u_relayout(u_bf[:, hs, :]),
            "ks2_sh": ks2,
        }
        m_.update(tabs)
        in_maps.append(m_)
    return in_maps


def kernel(u, A_re, A_im, BC_re, BC_im, D):
    from concourse.bass_utils import run_bass_kernel_spmd

    nc = _get_program()
    in_maps = make_in_maps(u, A_re, A_im, BC_re, BC_im, D)

    res = None
    last_err = None
    for attempt in range(3):
        try:
            res = run_bass_kernel_spmd(nc, in_maps, list(range(NCORES)))
            break
        except Exception as e:  # transient NRT_EXEC_UNIT_UNRECOVERABLE flakes
            last_err = e
            import time as _time
            _time.sleep(2.0)
    if res is None:
        raise last_err
    out = np.concatenate(
        [_y_relayout(res.results[c]["y2_sh"]) for c in range(NCORES)], axis=1)
    return np.ascontiguousarray(out, dtype=np.float32)


if __name__ == "__main__":
    rng = np.random.default_rng(0)
    u = rng.standard_normal((B, H, L), dtype=np.float32)
    A_re = rng.uniform(0.5, 0.99, P).astype(np.float32)
    A_im = rng.uniform(-0.5, 0.5, P).astype(np.float32)
    BC_re = rng.standard_normal((H, P), dtype=np.float32)
    BC_im = rng.standard_normal((H, P), dtype=np.float32)
    D = rng.uniform(0, 1, H).astype(np.float32)
    y = kernel(u=u, A_re=A_re, A_im=A_im, BC_re=BC_re, BC_im=BC_im, D=D)
    print("out", y.shape, y.dtype)


# revision 15
# speedup vs baseline: 77.3433x; 1.1029x over previous
"""Trainium2 Bass kernel for nn_FFTConv: y = tanh(Re(ifft(fft(u)*Ks)) + D*u).

v3 design:
  * Complex packing: conv with a REAL kernel commutes with Re/Im, so pack
    z[j] = u[2j] + i*u[2j+1] (j in [0,8)) per h -> halves all work. The
    real kernel's spectrum Ks (with D*u and the 1/L ifft scale folded in)
    is computed on the HOST from the 128 poles {A, conj(A)} and shipped as
    a bf16 parameter in its final on-chip layout -- no device prologue.
  * All matmuls bf16 (1 cyc/row), elementwise in bf16 SBUF (DVE 2x mode),
    full 128-partition layouts via block-diagonal stationaries for the
    64-point DFT stages. GPSIMD is avoided entirely (HW tensor ops there
    are ~20x slower than any model).
  * Fwd-twiddle add/sub folded into the corner-turn transposes via +/-I
    accumulation on the PE.
  * H-sharded over 8 cores (32 ch/core); per h: 8 packed complex rows,
    free dim 512 everywhere; NLANES software-pipelined lanes with skew.

Per-h dataflow (L = 8192 = 64*128, n = n1 + 128*n2, m = k2 + 64*k1):
  u:      [p=(g,n2), (c bb), n1]  one contiguous DMA (host pre-layout)
  stage1  (PE)  BD(F64) over n2        -> S1r,S1i [p=(g,k2), bb, n1]
  y1 copy (Act) PSUM->SBUF bf16 as [p, bb, (c n1)]
  fwd tw  (DVE) 2 paired mults ma=y1*[t2r|t2i], mb=y1*[t2i|t2r]
  transp  (PE)  16x 128x128, re = ma0.T - ma1.T, im = mb0.T + mb1.T
  y2t     (DVE) PSUM->SBUF bf16
  stage2  (PE)  F128 over n1           -> S2r,S2i [p=k1, (bb g), k2]
  x copy  (Act) PSUM->SBUF bf16 as [p, (bb g), (c k2)]
  spectral(DVE) 2 paired mults vs Ks[(r,i)|(i,r)] + 2 addsubs
  inv1    (PE)  conj(F128) over k1     -> Z1r,Z1i [p=o2, (bb g), k2]
  z1 copy (Act)
  inv tw  (DVE) 2 paired mults + 2 addsubs
  transp  (PE)  8x
  z2t     (Act) PSUM->SBUF bf16
  final   (PE)  BD(conj(F64)) over k2  -> Fr,Fi [p=(g,n2), bb, n1]
  tanh    (Act) from PSUM -> yo f32; one contiguous DMA out
"""
import os
import sys
import numpy as np

for p in ("/opt/trn_rl_repo", "/root/.axon_site/_ro/trn_rl_repo"):
    if os.path.isdir(p) and p not in sys.path:
        sys.path.append(p)

import ml_dtypes

BF16 = ml_dtypes.bfloat16

B, H, L, P = 16, 256, 8192, 64
NCORES = 8
HSH = H // NCORES          # 32 channels per core
NLANES = int(os.environ.get("KERNEL_NLANES", "4"))
REPEAT = int(os.environ.get("KERNEL_REPEAT", "1"))
MIDBUFS = int(os.environ.get("KERNEL_MIDBUFS", "2"))
IOBUFS = int(os.environ.get("KERNEL_IOBUFS", "2"))
PFBUFS = int(os.environ.get("KERNEL_PFBUFS", "2"))
PTBUFS = int(os.environ.get("KERNEL_PTBUFS", "0"))
SKEW = int(os.environ.get("KERNEL_SKEW", "2"))

_CACHE = {}


def _tables():
    a64 = np.arange(64)
    a128 = np.arange(128)
    c64 = np.cos(2 * np.pi * np.outer(a64, a64) / 64)
    s64 = np.sin(2 * np.pi * np.outer(a64, a64) / 64)
    c128 = np.cos(2 * np.pi * np.outer(a128, a128) / 128)
    s128 = np.sin(2 * np.pi * np.outer(a128, a128) / 128)
    z64 = np.zeros((64, 64))

    def bd(x):
        return np.block([[x, z64], [z64, x]])

    k2v = a128 % 64
    thT = 2 * np.pi * np.outer(k2v, a128) / L        # [(g,k2), n1]
    thI = 2 * np.pi * np.outer(a128, a64) / L        # [o2, k2]
    t2r, t2i = np.cos(thT), -np.sin(thT)
    tir, tii = np.cos(thI), np.sin(thI)
    t = {
        "bd64r": bd(c64), "bd64i": bd(-s64), "bd64in": bd(s64),
        "f128r": c128, "f128i": -s128, "f128in": s128,
        "i128": np.eye(128), "i128n": -np.eye(128),
        # paired twiddle tables: [re|im] and [im|re] side by side
        "t2a": np.concatenate([t2r, t2i], axis=1),   # [128, 256]
        "t2b": np.concatenate([t2i, t2r], axis=1),
        "tia": np.concatenate([tir, tii], axis=1),   # [128, 128]
        "tib": np.concatenate([tii, tir], axis=1),
    }
    return {k: v.astype(BF16) for k, v in t.items()}


TBL_SHAPES = {
    "bd64r": [128, 128], "bd64i": [128, 128], "bd64in": [128, 128],
    "f128r": [128, 128], "f128i": [128, 128], "f128in": [128, 128],
    "i128": [128, 128], "i128n": [128, 128],
    "t2a": [128, 256], "t2b": [128, 256],
    "tia": [128, 128], "tib": [128, 128],
}


def _build(nc_mod):
    bass, tile, mybir, bacc = nc_mod
    dt = mybir.dt
    f32 = dt.float32
    bf16 = dt.bfloat16

    nc = bacc.Bacc("TRN2", target_bir_lowering=False, debug=False)
    AF = mybir.ActivationFunctionType
    OP = mybir.AluOpType

    # ---------------- DRAM parameters ----------------
    # u2/y2 host-relayouted: [p=(g,n2), h, (c bb), n1] -> one contiguous
    # full-width DMA per h. ks2 is the host-computed spectrum in its final
    # on-chip layout [k1, h, (r,i,i,r), k2].
    u_d = nc.declare_dram_parameter("u2_sh", [128, HSH, 8, 128], bf16, isOutput=False)
    y_d = nc.declare_dram_parameter("y2_sh", [128, HSH, 8, 128], f32, isOutput=True)
    ks2_d = nc.declare_dram_parameter("ks2_sh", [128, HSH, 4, 64], bf16, isOutput=False)
    tbl_d = {n: nc.declare_dram_parameter(n, shp, bf16, isOutput=False)
             for n, shp in TBL_SHAPES.items()}

    with tile.TileContext(nc) as tc:
        with tc.tile_pool(name="const", bufs=1) as cpool:
            tb = {}
            for n in TBL_SHAPES:
                tb[n] = cpool.tile(TBL_SHAPES[n], bf16, tag=f"c_{n}", name=f"c_{n}")
                nc.sync.dma_start(tb[n][:], tbl_d[n][:])
            ks_all = cpool.tile([128, HSH, 4, 64], bf16, tag="ks_all")
            nc.sync.dma_start(
                ks_all[:].rearrange("p a b c -> p (a b c)"),
                ks2_d[:].rearrange("p a b c -> p (a b c)"))

            main_pools = [
                tc.tile_pool(name="io", bufs=IOBUFS),
                tc.tile_pool(name="mid", bufs=MIDBUFS),
                tc.tile_pool(name="pf", bufs=PFBUFS, space=bass.MemorySpace.PSUM),
            ]
            if PTBUFS > 0:
                main_pools.append(
                    tc.tile_pool(name="pt", bufs=PTBUFS, space=bass.MemorySpace.PSUM))
            iop = main_pools[0].__enter__()
            midp = main_pools[1].__enter__()
            pfp = main_pools[2].__enter__()
            ptp = main_pools[3].__enter__() if PTBUFS > 0 else None

            t2a_b = tb["t2a"][:].unsqueeze(1).broadcast_to([128, 4, 256])
            t2b_b = tb["t2b"][:].unsqueeze(1).broadcast_to([128, 4, 256])
            tia_b = tb["tia"][:].unsqueeze(1).broadcast_to([128, 8, 128])
            tib_b = tb["tib"][:].unsqueeze(1).broadcast_to([128, 8, 128])

            def stages(h, lane):
                sfx = str(lane)

                ks_a = ks_all[:, h, 0:2, :].rearrange("p a b -> p (a b)").unsqueeze(
                    1).broadcast_to([128, 8, 128])
                ks_b = ks_all[:, h, 2:4, :].rearrange("p a b -> p (a b)").unsqueeze(
                    1).broadcast_to([128, 8, 128])

                # --- load: one contiguous full-width DMA ---
                uc = iop.tile([128, 2, 512], bf16, tag="uc" + sfx, name="uc")
                nc.sync.dma_start(
                    uc[:].rearrange("p c (bb n1) -> p (c bb) n1", n1=128), u_d[:, h])
                yield

                # --- stage 1: BD(F64) over n2 ---
                S1i = pfp.tile([128, 512], f32, tag="pf" + sfx, name="S1i")
                S1r = pfp.tile([128, 512], f32, tag="pf" + sfx, name="S1r")
                ure, uim = uc[:, 0], uc[:, 1]
                nc.tensor.matmul(S1i[:], tb["bd64i"][:], ure, start=True, stop=False)
                nc.tensor.matmul(S1i[:], tb["bd64r"][:], uim, start=False, stop=True)
                nc.tensor.matmul(S1r[:], tb["bd64r"][:], ure, start=True, stop=False)
                nc.tensor.matmul(S1r[:], tb["bd64in"][:], uim, start=False, stop=True)
                # y1 layout [p, bb, (c n1)]
                y1 = midp.tile([128, 4, 256], bf16, tag="y1" + sfx, name="y1")
                nc.scalar.activation(
                    y1[:, :, 0:128], S1r[:].rearrange("p (a b) -> p a b", a=4), AF.Copy)
                nc.scalar.activation(
                    y1[:, :, 128:256], S1i[:].rearrange("p (a b) -> p a b", a=4), AF.Copy)
                yield

                # --- fwd twiddle: 2 paired mults + 2 addsubs (DVE) ---
                ma = midp.tile([128, 4, 256], bf16, tag="mA" + sfx, name="ma")
                mb = midp.tile([128, 4, 256], bf16, tag="mB" + sfx, name="mb")
                nc.vector.tensor_tensor(ma[:], y1[:], t2a_b, OP.mult)
                nc.vector.tensor_tensor(mb[:], y1[:], t2b_b, OP.mult)
                y2r = midp.tile([128, 512], bf16, tag="w1" + sfx, name="y2r")
                y2i = midp.tile([128, 512], bf16, tag="w2" + sfx, name="y2i")
                nc.vector.tensor_tensor(
                    y2r[:].rearrange("p (a b) -> p a b", a=4),
                    ma[:, :, 0:128], ma[:, :, 128:256], OP.subtract)
                nc.vector.tensor_tensor(
                    y2i[:].rearrange("p (a b) -> p a b", a=4),
                    mb[:, :, 0:128], mb[:, :, 128:256], OP.add)
                yield

                # --- fwd transposes ---
                TP = ptp.tile([128, 2, 512], bf16, tag="pt" + sfx, name="TP") \
                    if ptp is not None else \
                    pfp.tile([128, 2, 512], bf16, tag="pf" + sfx, name="TP")
                for cc, src in ((0, y2r), (1, y2i)):
                    tpv = TP[:, cc].rearrange("p (a b) -> p a b", a=4)
                    srcv = src[:].rearrange("p (a b) -> p a b", a=4)
                    for bb in range(4):
                        nc.tensor.transpose(tpv[:, bb, :], srcv[:, bb, :], tb["i128"][:])
                y2t = midp.tile([128, 2, 512], bf16, tag="y2t" + sfx, name="y2t")
                nc.vector.tensor_scalar_add(
                    y2t[:].rearrange("p a b -> p (a b)"),
                    TP[:].rearrange("p a b -> p (a b)"), 0.0)
                yield

                # --- stage 2: F128 over n1 ---
                S2i = pfp.tile([128, 512], f32, tag="pf" + sfx, name="S2i")
                S2r = pfp.tile([128, 512], f32, tag="pf" + sfx, name="S2r")
                y2tr, y2ti = y2t[:, 0], y2t[:, 1]
                nc.tensor.matmul(S2i[:], tb["f128i"][:], y2tr, start=True, stop=False)
                nc.tensor.matmul(S2i[:], tb["f128r"][:], y2ti, start=False, stop=True)
                nc.tensor.matmul(S2r[:], tb["f128r"][:], y2tr, start=True, stop=False)
                nc.tensor.matmul(S2r[:], tb["f128in"][:], y2ti, start=False, stop=True)
                # x layout [p, (bb g), (c k2)]
                x = midp.tile([128, 8, 128], bf16, tag="x" + sfx, name="x")
                nc.scalar.activation(
                    x[:, :, 0:64], S2r[:].rearrange("p (a b) -> p a b", a=8), AF.Copy)
                nc.scalar.activation(
                    x[:, :, 64:128], S2i[:].rearrange("p (a b) -> p a b", a=8), AF.Copy)
                yield

                # --- spectral multiply: 2 paired mults + 2 addsubs (DVE) ---
                ma = midp.tile([128, 8, 128], bf16, tag="mA" + sfx, name="ma")
                mb = midp.tile([128, 8, 128], bf16, tag="mB" + sfx, name="mb")
                nc.vector.tensor_tensor(ma[:], x[:], ks_a, OP.mult)
                nc.vector.tensor_tensor(mb[:], x[:], ks_b, OP.mult)
                sre = midp.tile([128, 512], bf16, tag="w1" + sfx, name="sre")
                sim_ = midp.tile([128, 512], bf16, tag="w2" + sfx, name="sim")
                s_re, s_im = sre[:], sim_[:]
                nc.vector.tensor_tensor(
                    sre[:].rearrange("p (a b) -> p a b", a=8),
                    ma[:, :, 0:64], ma[:, :, 64:128], OP.subtract)
                nc.vector.tensor_tensor(
                    sim_[:].rearrange("p (a b) -> p a b", a=8),
                    mb[:, :, 0:64], mb[:, :, 64:128], OP.add)
                yield

                # --- inverse stage 1: conj(F128) over k1 ---
                Z1i = pfp.tile([128, 512], f32, tag="pf" + sfx, name="Z1i")
                Z1r = pfp.tile([128, 512], f32, tag="pf" + sfx, name="Z1r")
                nc.tensor.matmul(Z1i[:], tb["f128in"][:], s_re, start=True, stop=False)
                nc.tensor.matmul(Z1i[:], tb["f128r"][:], s_im, start=False, stop=True)
                nc.tensor.matmul(Z1r[:], tb["f128r"][:], s_re, start=True, stop=False)
                nc.tensor.matmul(Z1r[:], tb["f128i"][:], s_im, start=False, stop=True)
                # z1 layout [p, (bb g), (c k2)]
                z1 = midp.tile([128, 8, 128], bf16, tag="z1" + sfx, name="z1")
                nc.scalar.activation(
                    z1[:, :, 0:64], Z1r[:].rearrange("p (a b) -> p a b", a=8), AF.Copy)
                nc.scalar.activation(
                    z1[:, :, 64:128], Z1i[:].rearrange("p (a b) -> p a b", a=8), AF.Copy)
                yield

                # --- inverse twiddle: 2 paired mults + 2 addsubs (DVE) ---
                ma = midp.tile([128, 8, 128], bf16, tag="mA" + sfx, name="ma")
                mb = midp.tile([128, 8, 128], bf16, tag="mB" + sfx, name="mb")
                nc.vector.tensor_tensor(ma[:], z1[:], tia_b, OP.mult)
                nc.vector.tensor_tensor(mb[:], z1[:], tib_b, OP.mult)
                z2r = midp.tile([128, 512], bf16, tag="w1" + sfx, name="z2r")
                z2i = midp.tile([128, 512], bf16, tag="w2" + sfx, name="z2i")
                nc.vector.tensor_tensor(
                    z2r[:].rearrange("p (a b) -> p a b", a=8),
                    ma[:, :, 0:64], ma[:, :, 64:128], OP.subtract)
                nc.vector.tensor_tensor(
                    z2i[:].rearrange("p (a b) -> p a b", a=8),
                    mb[:, :, 0:64], mb[:, :, 64:128], OP.add)
                yield

                # --- inverse transposes ---
                TQ = ptp.tile([128, 2, 512], bf16, tag="pt" + sfx, name="TQ") \
                    if ptp is not None else \
                    pfp.tile([128, 2, 512], bf16, tag="pf" + sfx, name="TQ")
                for cc, zsrc in ((0, z2r), (1, z2i)):
                    tqv = TQ[:, cc].rearrange("p (a b) -> p a b", a=4)
                    zsv = zsrc[:].rearrange("p (a b) -> p a b", a=4)
                    for bb in range(4):
                        nc.tensor.transpose(tqv[:, bb, :], zsv[:, bb, :], tb["i128"][:])
                z2t = midp.tile([128, 2, 512], bf16, tag="z2t" + sfx, name="z2t")
                nc.scalar.activation(
                    z2t[:].rearrange("p a b -> p (a b)"),
                    TQ[:].rearrange("p a b -> p (a b)"), AF.Copy)
                yield

                # --- final: BD(conj(F64)) over k2, tanh, store ---
                Fi = pfp.tile([128, 512], f32, tag="pf" + sfx, name="Fi")
                Fr = pfp.tile([128, 512], f32, tag="pf" + sfx, name="Fr")
                z2tr, z2ti = z2t[:, 0], z2t[:, 1]
                nc.tensor.matmul(Fi[:], tb["bd64in"][:], z2tr, start=True, stop=False)
                nc.tensor.matmul(Fi[:], tb["bd64r"][:], z2ti, start=False, stop=True)
                nc.tensor.matmul(Fr[:], tb["bd64r"][:], z2tr, start=True, stop=False)
                nc.tensor.matmul(Fr[:], tb["bd64i"][:], z2ti, start=False, stop=True)
                yo = iop.tile([128, 2, 512], f32, tag="yo" + sfx, name="yo")
                nc.scalar.activation(yo[:, 0], Fr[:], AF.Tanh)
                nc.scalar.activation(yo[:, 1], Fi[:], AF.Tanh)
                nc.sync.dma_start(
                    y_d[:, h], yo[:].rearrange("p c (bb n1) -> p (c bb) n1", n1=128))
                yield

            def lane_stream(ln):
                for _rep in range(REPEAT):
                    for h in range(ln, HSH, NLANES):
                        yield from stages(h, ln)

            gens = [lane_stream(ln) for ln in range(NLANES)]
            done = [False] * NLANES
            # prime lanes with a stage skew so engine queues interleave
            # different pipeline stages instead of running in lockstep
            for ln in range(NLANES):
                for _ in range((NLANES - 1 - ln) * SKEW):
                    try:
                        next(gens[ln])
                    except StopIteration:
                        done[ln] = True
                        break
            while not all(done):
                for gi_, g in enumerate(gens):
                    if not done[gi_]:
                        try:
                            next(g)
                        except StopIteration:
                            done[gi_] = True

            for mp in reversed(main_pools):
                mp.__exit__(None, None, None)

    nc.compile()
    return nc


def _get_program():
    key = ("prog", NLANES, REPEAT, MIDBUFS, IOBUFS, PFBUFS, PTBUFS, SKEW)
    if key not in _CACHE:
        import concourse.bass as bass
        import concourse.tile as tile
        from concourse import mybir, bacc
        _CACHE[key] = _build((bass, tile, mybir, bacc))
    return _CACHE[key]


def _u_relayout(u_bf_core):
    """[16, HSH, L] -> [p=(g,n2), h, (c bb), n1]  (b = g*8 + bb*2 + c)."""
    us = u_bf_core.reshape(2, 4, 2, HSH, 64, 128)   # g bb c h n2 n1
    return np.ascontiguousarray(us.transpose(0, 4, 3, 2, 1, 5).reshape(128, HSH, 8, 128))


def _y_relayout(y2_core):
    """[p=(g,n2), h, (c bb), n1] -> [16, HSH, L]."""
    ys = y2_core.reshape(2, 64, HSH, 2, 4, 128)      # g n2 h c bb n1
    return ys.transpose(0, 4, 3, 2, 1, 5).reshape(16, HSH, L)


def make_in_maps(u, A_re, A_im, BC_re, BC_im, D):
    tabs = _tables()
    u_bf = np.ascontiguousarray(u).astype(BF16)
    # host-side kernel spectrum: Ks[h,m] = (sum_p c'_p/(1-A'_p W^m) + D[h])/L
    m = np.arange(L)
    W = np.exp(-2j * np.pi * m / L).astype(np.complex64)
    A2 = np.concatenate([A_re + 1j * A_im, A_re - 1j * A_im]).astype(np.complex64)
    G = (1.0 / (1.0 - A2[:, None] * W[None, :])).astype(np.complex64)   # (128, L)
    BC = (BC_re + 1j * BC_im).astype(np.complex64)
    in_maps = []
    for c in range(NCORES):
        hs = slice(c * HSH, (c + 1) * HSH)
        C2 = np.concatenate([BC[hs] / 2, np.conj(BC[hs]) / 2], axis=1)  # (HSH, 128)
        Ks = (C2 @ G + D[hs].astype(np.complex64)[:, None]) / np.float32(L)  # (HSH, L)
        kr = Ks.real.astype(BF16).reshape(HSH, 128, 64)   # [h, k1, k2]
        ki = Ks.imag.astype(BF16).reshape(HSH, 128, 64)
        ks2 = np.stack([kr, ki, ki, kr], axis=2)          # [h, k1, c, k2]
        ks2 = np.ascontiguousarray(ks2.transpose(1, 0, 2, 3))  # [k1, h, c, k2]
        m_ = {
            "u2_sh": _u_relayout(u_bf[:, hs, :]),
            "ks2_sh": ks2,
        }
        m_.update(tabs)
        in_maps.append(m_)
    return in_maps


def kernel(u, A_re, A_im, BC_re, BC_im, D):
    from concourse.bass_utils import run_bass_kernel_spmd

    nc = _get_program()
    in_maps = make_in_maps(u, A_re, A_im, BC_re, BC_im, D)

    res = None
    last_err = None
    for attempt in range(3):
        try:
            res = run_bass_kernel_spmd(nc, in_maps, list(range(NCORES)))
            break
        except Exception as e:  # transient NRT_EXEC_UNIT_UNRECOVERABLE flakes
            last_err = e
            import time as _time
            _time.sleep(2.0)
    if res is None:
        raise last_err
    out = np.concatenate(
        [_y_relayout(res.results[c]["y2_sh"]) for c in range(NCORES)], axis=1)
    return np.ascontiguousarray(out, dtype=np.float32)


if __name__ == "__main__":
    rng = np.random.default_rng(0)
    u = rng.standard_normal((B, H, L), dtype=np.float32)
    A_re = rng.uniform(0.5, 0.99, P).astype(np.float32)
    A_im = rng.uniform(-0.5, 0.5, P).astype(np.float32)
    BC_re = rng.standard_normal((H, P), dtype=np.float32)
    BC_im = rng.standard_normal((H, P), dtype=np.float32)
    D = rng.uniform(0, 1, H).astype(np.float32)
    y = kernel(u=u, A_re=A_re, A_im=A_im, BC_re=BC_re, BC_im=BC_im, D=D)
    print("out", y.shape, y.dtype)
